# revision 1
# baseline (speedup 1.0000x reference)
"""MLA (multi-head latent attention) Trainium2 kernel, SPMD over 8 NeuronCores.

Sharding: core c = 4*b + j handles batch b and query rows [512j, 512j+512).
Each core computes the full K/V path for its batch (replicated within the
4-core batch group), attention + out-projection for its own query chunk.
No collectives; the host gather is a pure concat of disjoint output chunks.

All on-chip tensors live in transposed ([feature, token]) layouts so every
matmul contracts over the partition dim with no on-chip transposes:
  kv_latT[l,t] -> kcT/vT per head -> scoresT[k,q] -> exp -> ctxT[d,q] -> out[t,o]
rotate_half is folded into host-permuted weight copies; 1/sqrt(dh) into the
q weights; softmax skips the max-pass (scores bounded, exp cannot overflow)
and gets its row-sum from an all-ones matmul; normalization is fused into
the ctx PSUM evacuation.
"""

import contextlib
import os
import sys
import types

for _p in ("/opt/trn_rl_repo", "/root/.axon_site/_ro/trn_rl_repo"):
    if os.path.isdir(_p) and _p not in sys.path:
        sys.path.append(_p)

import numpy as np
import ml_dtypes

import concourse.bass as bass
import concourse.bacc as bacc_mod
import concourse.mybir as mybir
from concourse.tile import TileContext
from concourse.vector_clock import ScopedClock
from concourse.bass_utils import run_bass_kernel_spmd

F32 = mybir.dt.float32
BF16 = mybir.dt.bfloat16
BF16NP = ml_dtypes.bfloat16

HID, H, LAT, R, DH, C = 2048, 16, 512, 32, 128, 96
B, S = 2, 2048
SQ = 512          # query rows per core
NKC = S // 128    # 16 key chunks of 128
NG, GH = 4, 4     # 4 head-groups of 4 heads


def _patch_tile_drain():
    """The staged walrus rejects a Drain carrying >1 sync-wait. Move the
    TileContext tail-drain waits onto single-wait SP nops."""

    def _drain_and_barrier(self, tick_clock, wait_clock):
        drain_inst = self.nc.sync.drain()
        wait_clock.add_sem_waits(
            drain_inst.ins, ScopedClock({None: tick_clock.global_clock})
        )
        si = drain_inst.ins.sync_info
        if si is not None and len(si.on_wait) > 1:
            waits = list(si.on_wait)
            drain_inst.ins.sync_info = mybir.SyncInfo(
                on_wait=[], on_update=list(si.on_update)
            )
            for w in waits:
                nop = self.nc.sync.nop(nofuse=True)
                nop.ins.sync_info = mybir.SyncInfo(on_wait=[w], on_update=[])
        self.nc.all_engine_barrier()
        assert self.sems is not None
        popped = self.nc._tile_sem_poison_stack.pop()
        assert popped is self._sem_poison
        self.nc.clear_and_free_semaphores(list(self.sems.allocated().values()))
        self.nc.all_engine_barrier()

    TileContext._drain_and_barrier = _drain_and_barrier


def _install_ntff_hook():
    """antenv.axon_hooks is absent in this image; inject it and register the
    ctypes NTFF hook so trace=True / BASS_TRACE can profile."""
    try:
        import antenv

        if "antenv.axon_hooks" not in sys.modules:
            mod = types.ModuleType("antenv.axon_hooks")
            mod._hook = None

            def set_axon_ntff_profile_hook(h):
                mod._hook = h

            def get_axon_ntff_profile_hook():
                return mod._hook

            mod.set_axon_ntff_profile_hook = set_axon_ntff_profile_hook
            mod.get_axon_ntff_profile_hook = get_axon_ntff_profile_hook
            sys.modules["antenv.axon_hooks"] = mod
            antenv.axon_hooks = mod
        boot_dir = "/root/.axon_site/trn_agent_boot"
        so_path = "/opt/axon/libaxon_pjrt.so"
        if os.path.isdir(boot_dir) and os.path.exists(so_path):
            if boot_dir not in sys.path:
                sys.path.append(boot_dir)
            from trn_boot import _ntff_profile_via_ctypes

            hook = _ntff_profile_via_ctypes(so_path)
            if hook is not None:
                sys.modules["antenv.axon_hooks"].set_axon_ntff_profile_hook(hook)
    except Exception:
        pass


_patch_tile_drain()
_install_ntff_hook()


def _dram(nc, name, shape, dtype=F32, out=False):
    return nc.declare_dram_parameter(name, list(shape), dtype, isOutput=out)


def build_nc():
    nc = bacc_mod.Bacc("TRN2")

    xbT = _dram(nc, "xbT", [HID, S], BF16)            # x[b].T
    xqT = _dram(nc, "xqT", [HID, SQ], BF16)           # query-chunk rows of x[b], T
    wd_kvT = _dram(nc, "wd_kvT", [HID, LAT], BF16)    # Wkv_down.T
    wd_qT = _dram(nc, "wd_qT", [HID, LAT], BF16)      # Wq_down.T
    wkrT = _dram(nc, "wkrT", [HID, R], BF16)          # Wk_rope.T
    wkrrT = _dram(nc, "wkrrT", [HID, R], BF16)        # rot(Wk_rope).T
    wk_upT = _dram(nc, "wk_upT", [LAT, C * H], BF16)  # Wk_up.T
    wv_upT = _dram(nc, "wv_upT", [LAT, HID], BF16)    # Wv_up.T
    wqcrT = _dram(nc, "wqcrT", [LAT, 128 * H], BF16)  # per head: [Wq_up_h; Wq_rope_h].T / sqrt(DH)
    wqrrT = _dram(nc, "wqrrT", [LAT, 2 * R * H], BF16)    # per head: rot(Wq_rope_h).T / sqrt(DH)
    woT = _dram(nc, "woT", [HID, HID], BF16)    # Wo.T (bf16)
    bo_d = _dram(nc, "bo", [128, HID])  # host-broadcast
    cosqT_d = _dram(nc, "cosqT", [R, SQ])
    sinqT_d = _dram(nc, "sinqT", [R, SQ])
    coskT_d = _dram(nc, "coskT", [R, S], BF16)
    sinkT_d = _dram(nc, "sinkT", [R, S], BF16)
    maskT_d = _dram(nc, "maskT", [128, NKC * SQ], BF16)
    out_d = _dram(nc, "out", [SQ, HID], out=True)

    # [ (hc p) f ] views with 128-partition tiling of the contraction dim
    xbT_r2 = xbT[:, :].rearrange("(c p two) t -> c p two t", p=128, two=2)
    xqT_r2 = xqT[:, :].rearrange("(c p two) t -> c p two t", p=128, two=2)
    wd_kvT_r2 = wd_kvT[:, :].rearrange("(c p two) l -> c p two l", p=128, two=2)
    wd_qT_r2 = wd_qT[:, :].rearrange("(c p two) l -> c p two l", p=128, two=2)
    wkrT_r2 = wkrT[:, :].rearrange("(c p two) r -> c p two r", p=128, two=2)
    wkrrT_r2 = wkrrT[:, :].rearrange("(c p two) r -> c p two r", p=128, two=2)
    wk_upT_r = wk_upT[:, :].rearrange("(lc p) c -> lc p c", p=128)
    wv_upT_r = wv_upT[:, :].rearrange("(lc p) d -> lc p d", p=128)
    wqcrT_r = wqcrT[:, :].rearrange("(lc p) d -> lc p d", p=128)
    wqrrT_r = wqrrT[:, :].rearrange("(lc p) d -> lc p d", p=128)
    woT_r = woT[:, :].rearrange("(hc p) o -> hc p o", p=128)

    with TileContext(nc) as tc:
        with tc.tile_pool(name="perB", bufs=1) as perB:
            WO = None  # opened before phase 2; closed with perB
            ctxT = perB.tile([128, H, SQ], BF16, tag="ctxT", name="ctxT")  # [d%128, h, q]
            cosqT = perB.tile([32, SQ], F32, tag="cosq", name="cosq")
            sinqT = perB.tile([32, SQ], F32, tag="sinq", name="sinq")
            onesb = perB.tile([128, 128], BF16, tag="ones", name="ones")
            nc.sync.dma_start(cosqT[:], cosqT_d[:, :])
            nc.sync.dma_start(sinqT[:], sinqT_d[:, :])
            nc.gpsimd.memset(onesb[:], 1.0)

            with tc.tile_pool(name="perA", bufs=1) as perA:
                kv_latT = perA.tile([128, 4, S], BF16, tag="kvlat", name="kvlat")
                q_latT = perA.tile([128, 4, SQ], BF16, tag="qlat", name="qlat")
                krT = perA.tile([32, S], BF16, tag="krT", name="krT")
                maskT = perA.tile([128, NKC * SQ], BF16, tag="mask", name="mask")

                # ------------ Phase 1: latents + roped k_rope ------------
                with tc.tile_pool(name="w1", bufs=1) as W1, \
                     tc.tile_pool(name="xs", bufs=3) as XS, \
                     tc.tile_pool(name="tmp1", bufs=3) as T1, \
                     tc.tile_pool(name="ps_lat", bufs=1, space="PSUM") as PSL:
                    coskT = W1.tile([32, S], BF16, tag="cosk", name="cosk")
                    sinkT = W1.tile([32, S], BF16, tag="sink", name="sink")
                    wdkv = [W1.tile([128, 2, LAT], BF16, tag=f"wdkv{hch}",
                                    name=f"wdkv{hch}") for hch in range(8)]
                    wdq = [W1.tile([128, 2, LAT], BF16, tag=f"wdq{hch}",
                                   name=f"wdq{hch}") for hch in range(8)]
                    wkr = [W1.tile([128, 2, 2 * R], BF16, tag=f"wkr{hch}",
                                   name=f"wkr{hch}") for hch in range(8)]

                    # kv_latT + k_rope in one pass over xbT t-quarters
                    for tq in range(4):
                        tsl = slice(tq * 512, (tq + 1) * 512)
                        pss = [PSL.tile([128, 512], F32, tag=f"lat{lc}_0",
                                        name=f"lat{lc}") for lc in range(4)]
                        pkr = PSL.tile([64, 512], F32, tag="lat0_1",
                                       name="pkr")
                        for hch in range(8):
                            if tq == 0:
                                nc.sync.dma_start(wdkv[hch][:], wd_kvT_r2[hch])
                                nc.sync.dma_start(wkr[hch][:, :, 0:R],
                                                  wkrT_r2[hch])
                                nc.sync.dma_start(wkr[hch][:, :, R:2 * R],
                                                  wkrrT_r2[hch])
                            xbt = XS.tile([128, 2, 512], BF16, tag="xb",
                                          name="xb")
                            nc.sync.dma_start(xbt[:], xbT_r2[hch][:, :, tsl])
                            for two in range(2):
                                for lc in range(4):
                                    nc.tensor.matmul(
                                        pss[lc][:],
                                        lhsT=wdkv[hch][:, two,
                                                       lc * 128:(lc + 1) * 128],
                                        rhs=xbt[:, two, :],
                                        start=(hch == 0 and two == 0),
                                        stop=(hch == 7 and two == 1),
                                    )
                                nc.tensor.matmul(
                                    pkr[:],
                                    lhsT=wkr[hch][:, two, :],
                                    rhs=xbt[:, two, :],
                                    start=(hch == 0 and two == 0),
                                    stop=(hch == 7 and two == 1),
                                )
                        if tq == 0:
                            nc.sync.dma_start(coskT[:], coskT_d[:, :])
                            nc.sync.dma_start(sinkT[:], sinkT_d[:, :])
                        for lc in range(4):
                            nc.scalar.copy(kv_latT[:, lc, tsl], pss[lc][:])
                        t1 = T1.tile([32, 512], F32, tag="r1", name="r1")
                        t2_ = T1.tile([32, 512], F32, tag="r2", name="r2")
                        nc.vector.tensor_mul(t1[:], pkr[0:32, :], coskT[:, tsl])
                        nc.vector.tensor_mul(t2_[:], pkr[32:64, :],
                                             sinkT[:, tsl])
                        nc.vector.tensor_add(krT[:, tsl], t1[:], t2_[:])

                    # q_latT[l, q] over the core's own query chunk
                    psq = [PSL.tile([128, 512], F32, tag=f"lat{lc}_0",
                                    name=f"q{lc}") for lc in range(4)]
                    for hch in range(8):
                        nc.sync.dma_start(wdq[hch][:], wd_qT_r2[hch])
                        xqt = XS.tile([128, 2, SQ], BF16, tag="xq", name="xq")
                        nc.sync.dma_start(xqt[:], xqT_r2[hch])
                        for two in range(2):
                            for lc in range(4):
                                nc.tensor.matmul(
                                    psq[lc][:],
                                    lhsT=wdq[hch][:, two,
                                                  lc * 128:(lc + 1) * 128],
                                    rhs=xqt[:, two, :],
                                    start=(hch == 0 and two == 0),
                                    stop=(hch == 7 and two == 1),
                                )
                    for lc in range(4):
                        nc.scalar.copy(q_latT[:, lc, :], psq[lc][:])

                # ------------ Phase 2: per head-group proj + attention ----
                nc.sync.dma_start(maskT[:], maskT_d[:, :])

                with tc.tile_pool(name="grp", bufs=2) as GRP, \
                     tc.tile_pool(name="gw", bufs=2) as GW, \
                     tc.tile_pool(name="et", bufs=2) as ETP, \
                     tc.tile_pool(name="tmp2", bufs=2) as T2, \
                     tc.tile_pool(name="rcp", bufs=2) as RCP, \
                     tc.tile_pool(name="ps_p", bufs=1, space="PSUM") as PSP, \
                     tc.tile_pool(name="ps_s", bufs=3, space="PSUM") as PSS, \
                     tc.tile_pool(name="ps_c", bufs=2, space="PSUM") as PSC, \
                     tc.tile_pool(name="ps_r", bufs=1, space="PSUM") as PSR:
                    for g in range(NG):
                        wk_g = GW.tile([128, 4, GH * C], BF16, tag="wk", name="wk")
                        wv_g = GW.tile([128, 4, GH * DH], BF16, tag="wv", name="wv")
                        wq_g = GW.tile([128, 4, GH * 128], BF16, tag="wq", name="wq")
                        wqr_g = GW.tile([128, 4, GH * 2 * R], BF16, tag="wqr", name="wqr")
                        for lc in range(4):
                            nc.sync.dma_start(
                                wk_g[:, lc, :],
                                wk_upT_r[lc][:, g * GH * C:(g + 1) * GH * C],
                            )
                            nc.sync.dma_start(
                                wv_g[:, lc, :],
                                wv_upT_r[lc][:, g * GH * DH:(g + 1) * GH * DH],
                            )
                            nc.sync.dma_start(
                                wq_g[:, lc, :],
                                wqcrT_r[lc][:, g * GH * 128:(g + 1) * GH * 128],
                            )
                            nc.sync.dma_start(
                                wqr_g[:, lc, :],
                                wqrrT_r[lc][:, g * GH * 2 * R:(g + 1) * GH * 2 * R],
                            )

                        kT_g = GRP.tile([128, GH, S], BF16, tag="kT", name="kT")
                        v_g = GRP.tile([128, NKC, GH * DH], BF16, tag="vG", name="vG")
                        qT_g = GRP.tile([128, GH, SQ], BF16, tag="qT", name="qT")

                        # k content rows [0:96], shared roped k_rope rows [96:128]
                        for hh in range(GH):
                            for half in range(2):
                                pk = [PSP.tile([128, 512], F32, tag=("pa", "pb")[t2],
                                               name=f"pk{t2}")
                                      for t2 in range(2)]
                                for lc in range(4):
                                    for t2 in range(2):
                                        nc.tensor.matmul(
                                            pk[t2][0:C, :],
                                            lhsT=wk_g[:, lc, hh * C:(hh + 1) * C],
                                            rhs=kv_latT[
                                                :, lc,
                                                (half * 2 + t2) * 512:
                                                (half * 2 + t2 + 1) * 512,
                                            ],
                                            start=(lc == 0), stop=(lc == 3),
                                        )
                                for t2 in range(2):
                                    t0 = (half * 2 + t2) * 512
                                    nc.scalar.copy(
                                        kT_g[0:C, hh, t0:t0 + 512],
                                        pk[t2][0:C, :],
                                    )
                            nc.sync.dma_start(kT_g[C:128, hh, :], krT[:, :])

                        # v[t, d] for the group 4 heads
                        for kc in range(NKC):
                            pv = PSP.tile([128, 512], F32, tag="pa", name="pv")
                            for lc in range(4):
                                nc.tensor.matmul(
                                    pv[:],
                                    lhsT=kv_latT[:, lc, kc * 128:(kc + 1) * 128],
                                    rhs=wv_g[:, lc, :],
                                    start=(lc == 0), stop=(lc == 3),
                                )
                            nc.scalar.copy(v_g[:, kc, :], pv[:])

                        # q: content + roped rope rows
                        for hh in range(GH):
                            pqc = PSP.tile([96, 512], F32, tag="pa", name="pqc")
                            pqr2 = PSP.tile([64, 512], F32, tag="pb", name="pqr2")
                            for lc in range(4):
                                nc.tensor.matmul(
                                    pqc[:],
                                    lhsT=wq_g[:, lc, hh * 128:hh * 128 + C],
                                    rhs=q_latT[:, lc, :],
                                    start=(lc == 0), stop=(lc == 3),
                                )
                                nc.tensor.matmul(
                                    pqr2[:],
                                    lhsT=wqr_g[:, lc, hh * 2 * R:(hh + 1) * 2 * R],
                                    rhs=q_latT[:, lc, :],
                                    start=(lc == 0), stop=(lc == 3),
                                )
                            nc.scalar.copy(qT_g[0:C, hh, :], pqc[:])
                            t1 = T2.tile([32, SQ], F32, tag="r1", name="t1")
                            t2_ = T2.tile([32, SQ], F32, tag="r2", name="t2")
                            t3 = T2.tile([32, SQ], BF16, tag="r3", name="t3")
                            nc.vector.tensor_mul(t1[:], pqr2[0:32, :], cosqT[:])
                            nc.vector.tensor_mul(t2_[:], pqr2[32:64, :], sinqT[:])
                            nc.vector.tensor_add(t3[:], t1[:], t2_[:])
                            nc.sync.dma_start(qT_g[C:128, hh, :], t3[:])

                        # attention for the group heads
                        for hh in range(GH):
                            h = g * GH + hh
                            ets = []
                            for kc in range(NKC):
                                ps = PSS.tile([128, 512], F32, tag="s",
                                              name=f"ps{kc % 2}")
                                nc.tensor.matmul(
                                    ps[:],
                                    lhsT=kT_g[:, hh, kc * 128:(kc + 1) * 128],
                                    rhs=qT_g[:, hh, :],
                                    start=True, stop=True,
                                )
                                nc.vector.tensor_add(
                                    ps[:], ps[:],
                                    maskT[:, kc * SQ:(kc + 1) * SQ],
                                )
                                et = ETP.tile([128, SQ], BF16, tag=f"e{kc}",
                                              name=f"et{kc}")
                                nc.scalar.activation(
                                    et[:], ps[:],
                                    mybir.ActivationFunctionType.Exp,
                                )
                                ets.append(et)
                            pctx = PSC.tile([128, 512], F32, tag="c",
                                            name=f"pctx{hh % 2}")
                            prs = PSR.tile([128, 512], F32, tag="r",
                                           name=f"prs{hh % 2}")
                            for kc in range(NKC):
                                nc.tensor.matmul(
                                    pctx[:],
                                    lhsT=v_g[:, kc, hh * DH:(hh + 1) * DH],
                                    rhs=ets[kc][:],
                                    start=(kc == 0), stop=(kc == NKC - 1),
                                )
                            # in-place DVE tree-sum of the exp tiles,
                            # then a single all-ones matmul for the row-sum
                            step = 1
                            while step < NKC:
                                for i in range(0, NKC, 2 * step):
                                    nc.vector.tensor_add(
                                        ets[i][:], ets[i][:], ets[i + step][:]
                                    )
                                step *= 2
                            nc.tensor.matmul(
                                prs[:], lhsT=onesb[:], rhs=ets[0][:],
                                start=True, stop=True,
                            )
                            rc = RCP.tile([128, 512], F32, tag="rc",
                                          name=f"rc{hh % 2}")
                            nc.vector.reciprocal_approx_fast(out=rc[:], in_=prs[:])
                            nc.vector.tensor_mul(ctxT[:, h, :], pctx[:], rc[:])

            # ---------------- Phase 3: output projection ------------------
            with tc.tile_pool(name="op", bufs=2) as OP, \
                 tc.tile_pool(name="ps_o", bufs=2, space="PSUM") as PSO:
                WO = tc.alloc_tile_pool(name="wo", bufs=1, side="right")
                wo_sb = [WO.tile([128, HID], BF16, tag=f"wo{hc}",
                                 name=f"wo{hc}") for hc in range(16)]
                bo_sb = WO.tile([128, HID], F32, tag="bo", name="bo")
                nc.sync.dma_start(bo_sb[:], bo_d[:, :])
                for hc in range(16):
                    nc.sync.dma_start(wo_sb[hc][:], woT_r[hc])
                for tq in range(4):
                    pos = [PSO.tile([128, 512], F32, tag=f"o{oc}",
                                    name=f"pos{oc}") for oc in range(4)]
                    for h in range(16):
                        for oc in range(4):
                            nc.tensor.matmul(
                                pos[oc][:],
                                lhsT=ctxT[:, h, tq * 128:(tq + 1) * 128],
                                rhs=wo_sb[h][:, oc * 512:(oc + 1) * 512],
                                start=(h == 0), stop=(h == 15),
                            )
                    ot = OP.tile([128, HID], F32, tag="ot", name="ot")
                    for oc in range(4):
                        nc.vector.tensor_add(
                            ot[:, oc * 512:(oc + 1) * 512],
                            pos[oc][:],
                            bo_sb[:, oc * 512:(oc + 1) * 512],
                        )
                    nc.sync.dma_start(
                        out_d[tq * 128:(tq + 1) * 128, :], ot[:]
                    )
                WO.release()

    nc.compile()
    return nc


def _rot_rows(w):
    # rows of w are the rope dim; rot(w) @ lat == rotate_half(w @ lat)
    hR = w.shape[0] // 2
    return np.concatenate([-w[hR:], w[:hR]], axis=0)


def _prep_inputs(inputs):
    x = np.asarray(inputs["x"], np.float32)
    Wq_down = np.asarray(inputs["Wq_down"], np.float32)
    Wq_up = np.asarray(inputs["Wq_up"], np.float32)
    Wq_rope = np.asarray(inputs["Wq_rope"], np.float32)
    Wkv_down = np.asarray(inputs["Wkv_down"], np.float32)
    Wk_up = np.asarray(inputs["Wk_up"], np.float32)
    Wk_rope = np.asarray(inputs["Wk_rope"], np.float32)
    Wv_up = np.asarray(inputs["Wv_up"], np.float32)
    Wo = np.asarray(inputs["Wo"], np.float32)
    bo = np.asarray(inputs["bo"], np.float32)

    s = np.float32(1.0 / np.sqrt(DH))

    wd_kvT = np.ascontiguousarray(Wkv_down.T).astype(BF16NP)
    wd_qT = np.ascontiguousarray(Wq_down.T).astype(BF16NP)
    wkrT = np.ascontiguousarray(Wk_rope.T).astype(BF16NP)
    wkrrT = np.ascontiguousarray(_rot_rows(Wk_rope).T).astype(BF16NP)
    wk_upT = np.ascontiguousarray(Wk_up.T).astype(BF16NP)
    wv_upT = np.ascontiguousarray(Wv_up.T).astype(BF16NP)

    wqcr = np.empty((128 * H, LAT), np.float32)
    wqrr = np.empty((2 * R * H, LAT), np.float32)
    for h in range(H):
        wqcr[h * 128:h * 128 + C] = Wq_up[h * C:(h + 1) * C] * s
        wqcr[h * 128 + C:(h + 1) * 128] = Wq_rope[h * R:(h + 1) * R] * s
        wqrr[h * 2 * R:h * 2 * R + R] = Wq_rope[h * R:(h + 1) * R] * s
        wqrr[h * 2 * R + R:(h + 1) * 2 * R] = _rot_rows(Wq_rope[h * R:(h + 1) * R]) * s
    wqcrT = np.ascontiguousarray(wqcr.T).astype(BF16NP)
    wqrrT = np.ascontiguousarray(wqrr.T).astype(BF16NP)
    woT = np.ascontiguousarray(Wo.T).astype(BF16NP)
    bo2 = np.ascontiguousarray(np.broadcast_to(bo.reshape(1, HID), (128, HID)))

    inv_freq = (1.0 / (10000.0 ** (np.arange(0, R, 2, dtype=np.float32) / R)))
    t = np.arange(S, dtype=np.float32)
    freqs = t[:, None] * inv_freq[None, :]
    emb = np.concatenate([freqs, freqs], axis=-1)          # [S, R]
    cos = np.cos(emb).astype(np.float32)
    sin = np.sin(emb).astype(np.float32)
    coskT = np.ascontiguousarray(cos.T)
    sinkT = np.ascontiguousarray(sin.T)

    kar = np.arange(128)[:, None]
    qar = np.arange(SQ)[None, :]

    in_maps = []
    for c in range(8):
        b, j = divmod(c, 4)
        q0 = j * SQ
        maskT = np.empty((128, NKC * SQ), np.float32)
        for kc in range(NKC):
            vis = (kc * 128 + kar) <= (q0 + qar)
            maskT[:, kc * SQ:(kc + 1) * SQ] = np.where(vis, 0.0, -10000.0)
        in_maps.append({
            "xbT": np.ascontiguousarray(x[b].T).astype(BF16NP),
            "xqT": np.ascontiguousarray(x[b, q0:q0 + SQ].T).astype(BF16NP),
            "wd_kvT": wd_kvT, "wd_qT": wd_qT,
            "wkrT": wkrT, "wkrrT": wkrrT,
            "wk_upT": wk_upT, "wv_upT": wv_upT,
            "wqcrT": wqcrT, "wqrrT": wqrrT,
            "woT": woT, "bo": bo2,
            "cosqT": np.ascontiguousarray(cos[q0:q0 + SQ].T),
            "sinqT": np.ascontiguousarray(sin[q0:q0 + SQ].T),
            "coskT": coskT.astype(BF16NP), "sinkT": sinkT.astype(BF16NP),
            "maskT": maskT.astype(BF16NP),
        })
    return in_maps


_NC_CACHE = None


def kernel(**inputs):
    global _NC_CACHE
    if _NC_CACHE is None:
        _NC_CACHE = build_nc()
    nc = _NC_CACHE
    in_maps = _prep_inputs(inputs)
    res = run_bass_kernel_spmd(nc, in_maps, list(range(8)))
    out = np.empty((B, S, HID), np.float32)
    for c in range(8):
        b, j = divmod(c, 4)
        out[b, j * SQ:(j + 1) * SQ] = res.results[c]["out"]
    return out



# revision 10
# speedup vs baseline: 1.3105x; 1.3105x over previous
"""MLA (multi-head latent attention) Trainium2 kernel, SPMD over 8 NeuronCores.

Sharding: core c = 4*b + g handles batch b and head group g (4 heads),
ALL 2048 query rows.  Causality: query chunk c (512 rows) only attends
key chunks 0..4c+3 (lower triangle), so every core does the same
triangular work -- perfectly balanced, no masks off the diagonal.
Each core emits a PARTIAL out-projection (contraction over its 4 heads'
128-dims); the host sums the 4 partials per batch (+bias).  No
collectives.

On-chip layouts are transposed ([feature, token]) so every matmul
contracts over the partition dim with no on-chip transposes.
rotate_half is folded into host-permuted weight copies; 1/sqrt(dh) into
the q weights; softmax skips the max-pass (scores bounded) and gets its
row-sum from an all-ones matmul over a DVE tree-sum of the exp tiles.
Diagonal score tiles are masked multiplicatively (0/1 bf16) after exp.
"""

import os
import sys
import types

for _p in ("/opt/trn_rl_repo", "/root/.axon_site/_ro/trn_rl_repo"):
    if os.path.isdir(_p) and _p not in sys.path:
        sys.path.append(_p)

import numpy as np
import ml_dtypes

import concourse.bass as bass
import concourse.bacc as bacc_mod
import concourse.mybir as mybir
from concourse.tile import TileContext
from concourse.vector_clock import ScopedClock
from concourse.bass_utils import run_bass_kernel_spmd

F32 = mybir.dt.float32
BF16 = mybir.dt.bfloat16
BF16NP = ml_dtypes.bfloat16

HID, H, LAT, R, DH, C = 2048, 16, 512, 32, 128, 96
B, S = 2, 2048
GH = 4            # heads per core
NQC = 4           # query chunks of 512
NKC = 16          # key chunks of 128


def _patch_tile_drain():
    """The staged walrus rejects a Drain carrying >1 sync-wait. Move the
    TileContext tail-drain waits onto single-wait SP nops."""

    def _drain_and_barrier(self, tick_clock, wait_clock):
        drain_inst = self.nc.sync.drain()
        wait_clock.add_sem_waits(
            drain_inst.ins, ScopedClock({None: tick_clock.global_clock})
        )
        si = drain_inst.ins.sync_info
        if si is not None and len(si.on_wait) > 1:
            waits = list(si.on_wait)
            drain_inst.ins.sync_info = mybir.SyncInfo(
                on_wait=[], on_update=list(si.on_update)
            )
            for w in waits:
                nop = self.nc.sync.nop(nofuse=True)
                nop.ins.sync_info = mybir.SyncInfo(on_wait=[w], on_update=[])
        self.nc.all_engine_barrier()
        assert self.sems is not None
        popped = self.nc._tile_sem_poison_stack.pop()
        assert popped is self._sem_poison
        self.nc.clear_and_free_semaphores(list(self.sems.allocated().values()))
        self.nc.all_engine_barrier()

    TileContext._drain_and_barrier = _drain_and_barrier


def _install_ntff_hook():
    """antenv.axon_hooks is absent in this image; inject it and register the
    ctypes NTFF hook so trace=True / BASS_TRACE can profile."""
    try:
        import antenv

        if "antenv.axon_hooks" not in sys.modules:
            mod = types.ModuleType("antenv.axon_hooks")
            mod._hook = None

            def set_axon_ntff_profile_hook(h):
                mod._hook = h

            def get_axon_ntff_profile_hook():
                return mod._hook

            mod.set_axon_ntff_profile_hook = set_axon_ntff_profile_hook
            mod.get_axon_ntff_profile_hook = get_axon_ntff_profile_hook
            sys.modules["antenv.axon_hooks"] = mod
            antenv.axon_hooks = mod
        boot_dir = "/root/.axon_site/trn_agent_boot"
        so_path = "/opt/axon/libaxon_pjrt.so"
        if os.path.isdir(boot_dir) and os.path.exists(so_path):
            if boot_dir not in sys.path:
                sys.path.append(boot_dir)
            from trn_boot import _ntff_profile_via_ctypes

            hook = _ntff_profile_via_ctypes(so_path)
            if hook is not None:
                sys.modules["antenv.axon_hooks"].set_axon_ntff_profile_hook(hook)
    except Exception:
        pass


_patch_tile_drain()
_install_ntff_hook()


def _dram(nc, name, shape, dtype=F32, out=False):
    return nc.declare_dram_parameter(name, list(shape), dtype, isOutput=out)


def build_nc():
    nc = bacc_mod.Bacc("TRN2")

    xbT = _dram(nc, "xbT", [HID, S], BF16)            # x[b].T
    wd_kvT = _dram(nc, "wd_kvT", [HID, LAT], BF16)    # Wkv_down.T
    wd_qT = _dram(nc, "wd_qT", [HID, LAT], BF16)      # Wq_down.T
    wkr2T = _dram(nc, "wkr2T", [HID, 2 * R], BF16)    # [Wk_rope; rot].T
    wk_pT = _dram(nc, "wk_pT", [LAT, GH * C], BF16)   # 4-head k_c pack .T
    wv_pT = _dram(nc, "wv_pT", [LAT, GH * DH], BF16)  # 4-head v pack .T
    wqc_pT = _dram(nc, "wqc_pT", [LAT, GH * C], BF16)   # 4-head q_c pack /sqrt
    wqr_pT = _dram(nc, "wqr_pT", [LAT, GH * R], BF16)   # 4-head q_rope /sqrt
    wqrr_pT = _dram(nc, "wqrr_pT", [LAT, GH * R], BF16)  # rotated rope /sqrt
    woT = _dram(nc, "woT", [GH * DH, HID], BF16)      # Wo cols for our heads
    cos4_d = _dram(nc, "cos4", [128, S], BF16)        # cos.T tiled 4x
    sin4_d = _dram(nc, "sin4", [128, S], BF16)
    mask4_d = _dram(nc, "mask4", [128, NQC * 512], BF16)  # 0/1 diag masks
    out_d = _dram(nc, "out", [S, HID], out=True)      # partial (4-head) proj

    xbT_r = xbT[:, :].rearrange("(c p two) t -> c p two t", p=128, two=2)
    wd_kvT_r = wd_kvT[:, :].rearrange("(c p two) l -> c p two l", p=128, two=2)
    wd_qT_r = wd_qT[:, :].rearrange("(c p two) l -> c p two l", p=128, two=2)
    wkr2T_r = wkr2T[:, :].rearrange("(c p two) r -> c p two r", p=128, two=2)
    wk_pT_r = wk_pT[:, :].rearrange("(lc p) d -> lc p d", p=128)
    wv_pT_r = wv_pT[:, :].rearrange("(lc p) d -> lc p d", p=128)
    wqc_pT_r = wqc_pT[:, :].rearrange("(lc p) d -> lc p d", p=128)
    wqr_pT_r = wqr_pT[:, :].rearrange("(lc p) d -> lc p d", p=128)
    wqrr_pT_r = wqrr_pT[:, :].rearrange("(lc p) d -> lc p d", p=128)
    woT_r = woT[:, :].rearrange("(hc p) o -> hc p o", p=128)

    with TileContext(nc) as tc:
        with tc.tile_pool(name="perB", bufs=1) as perB, \
             tc.tile_pool(name="lat", bufs=2) as LATP, \
             tc.tile_pool(name="xs", bufs=1) as XS, \
             tc.tile_pool(name="ets", bufs=6) as ETS, \
             tc.tile_pool(name="acc", bufs=8) as ACC, \
             tc.tile_pool(name="rcp", bufs=2) as RCP, \
             tc.tile_pool(name="tmp", bufs=2) as TMP, \
             tc.tile_pool(name="ot", bufs=3) as OT, \
             tc.tile_pool(name="ps_g", bufs=2, space="PSUM") as PSG, \
             tc.tile_pool(name="ps_m", bufs=3, space="PSUM") as PSM, \
             tc.tile_pool(name="ps_s", bufs=2, space="PSUM") as PSS, \
             tc.tile_pool(name="ps_c", bufs=1, space="PSUM") as PSC:

            # ---------- persistent SBUF ----------
            krT = perB.tile([32, S], BF16, tag="krT", name="krT")
            kT = perB.tile([128, GH, S], BF16, tag="kT", name="kT")
            vG = perB.tile([128, NKC, GH * DH], BF16, tag="vG", name="vG")
            qT = perB.tile([128, GH, S], BF16, tag="qT", name="qT")
            ctxT = perB.tile([128, GH, S], BF16, tag="ctxT", name="ctxT")
            cos4 = perB.tile([128, S], BF16, tag="cos4", name="cos4")
            sin4 = perB.tile([128, S], BF16, tag="sin4", name="sin4")
            mask4 = perB.tile([128, NQC * 512], BF16, tag="mask4", name="mask4")
            onesb = perB.tile([128, 128], BF16, tag="ones", name="ones")
            wk_sb = perB.tile([128, 4, GH * C], BF16, tag="wk", name="wk")
            wv_sb = perB.tile([128, 4, GH * DH], BF16, tag="wv", name="wv")
            wqc_sb = perB.tile([128, 4, GH * C], BF16, tag="wqc", name="wqc")
            wqr_sb = perB.tile([128, 4, GH * R], BF16, tag="wqr", name="wqr")
            wqrr_sb = perB.tile([128, 4, GH * R], BF16, tag="wqrr", name="wqrr")

            nc.sync.dma_start(cos4[:], cos4_d[:, :])
            nc.sync.dma_start(sin4[:], sin4_d[:, :])
            nc.sync.dma_start(mask4[:], mask4_d[:, :])
            nc.gpsimd.memset(onesb[:], 1.0)
            for lc in range(4):
                nc.sync.dma_start(wk_sb[:, lc, :], wk_pT_r[lc])
                nc.sync.dma_start(wv_sb[:, lc, :], wv_pT_r[lc])
                nc.sync.dma_start(wqc_sb[:, lc, :], wqc_pT_r[lc])
                nc.sync.dma_start(wqr_sb[:, lc, :], wqr_pT_r[lc])
                nc.sync.dma_start(wqrr_sb[:, lc, :], wqrr_pT_r[lc])

            # down-proj weights: released after phase A(3), wo loaded after
            WD = tc.alloc_tile_pool(name="wd", bufs=1, side="right")
            wdkv = WD.tile([128, 8, 2, LAT], BF16, tag="wdkv", name="wdkv")
            wdq = WD.tile([128, 8, 2, LAT], BF16, tag="wdq", name="wdq")
            wkr = WD.tile([128, 8, 2, 2 * R], BF16, tag="wkr", name="wkr")
            for hc in range(8):
                nc.sync.dma_start(wdkv[:, hc, :, :], wd_kvT_r[hc])
                nc.sync.dma_start(wdq[:, hc, :, :], wd_qT_r[hc])
                nc.sync.dma_start(wkr[:, hc, :, :], wkr2T_r[hc])
            WO = [None]  # box for the late wo pool
            wo_sb = [None]

            # ---------------- phase emitters ----------------
            def phA(tq):
                """latents for token quarter tq: kv_lat, roped k_rope, q_lat.
                Returns the per-quarter latent tiles for phB(tq)."""
                tsl = slice(tq * 512, (tq + 1) * 512)
                xt = XS.tile([128, 8, 2, 512], BF16, tag="xf", name="xf")
                for hc in range(8):
                    nc.sync.dma_start(xt[:, hc, :, :], xbT_r[hc][:, :, tsl])
                kv_t = LATP.tile([128, 4, 512], BF16, tag="kvlat",
                                 name="kvlat")
                q_t = LATP.tile([128, 4, 512], BF16, tag="qlat", name="qlat")

                # kv_lat: 4 lc passes, 2 rotating psum banks
                for lc in range(4):
                    ps = PSG.tile([128, 512], F32, tag="g", name=f"pkv{lc}")
                    for hc in range(8):
                        for two in range(2):
                            nc.tensor.matmul(
                                ps[:],
                                lhsT=wdkv[:, hc, two, lc * 128:(lc + 1) * 128],
                                rhs=xt[:, hc, two, :],
                                start=(hc == 0 and two == 0),
                                stop=(hc == 7 and two == 1),
                            )
                    nc.vector.tensor_copy(kv_t[:, lc, :], ps[:])
                # k_rope pass (64 rows: [rope; rot]); combine in place
                pkr = PSG.tile([64, 512], F32, tag="g", name="pkr")
                for hc in range(8):
                    for two in range(2):
                        nc.tensor.matmul(
                            pkr[:],
                            lhsT=wkr[:, hc, two, :],
                            rhs=xt[:, hc, two, :],
                            start=(hc == 0 and two == 0),
                            stop=(hc == 7 and two == 1),
                        )
                nc.vector.tensor_mul(pkr[0:32, :], pkr[0:32, :],
                                     cos4[0:32, tsl])
                tkr = TMP.tile([32, 512], F32, tag="tkr", name="tkr")
                nc.vector.tensor_mul(tkr[:], pkr[32:64, :], sin4[0:32, tsl])
                nc.vector.tensor_add(krT[:, tsl], pkr[0:32, :], tkr[:])
                # q_lat: 4 lc passes
                for lc in range(4):
                    ps = PSG.tile([128, 512], F32, tag="g", name=f"pq{lc}")
                    for hc in range(8):
                        for two in range(2):
                            nc.tensor.matmul(
                                ps[:],
                                lhsT=wdq[:, hc, two, lc * 128:(lc + 1) * 128],
                                rhs=xt[:, hc, two, :],
                                start=(hc == 0 and two == 0),
                                stop=(hc == 7 and two == 1),
                            )
                    nc.vector.tensor_copy(q_t[:, lc, :], ps[:])
                return kv_t, q_t

            def phB(tq, kv_t, q_t):
                """per-head projections for quarter tq: k_c, v, q_c, q_rope."""
                tsl = slice(tq * 512, (tq + 1) * 512)
                # k_c per head (96 content rows)
                for h in range(GH):
                    ps = PSM.tile([128, 512], F32, tag="m", name=f"pk{h}")
                    for lc in range(4):
                        nc.tensor.matmul(
                            ps[0:C, :],
                            lhsT=wk_sb[:, lc, h * C:(h + 1) * C],
                            rhs=kv_t[:, lc, :],
                            start=(lc == 0), stop=(lc == 3),
                        )
                    nc.vector.tensor_copy(kT[0:C, h, tsl], ps[0:C, :])
                # shared roped k_rope rows -> kT[96:128] per head (DMA)
                for h in range(GH):
                    nc.sync.dma_start(kT[C:128, h, tsl], krT[:, tsl])
                # v: 4 token sub-chunks of 128, out = [t, 4h*128]
                for t2 in range(4):
                    kc = tq * 4 + t2
                    ps = PSM.tile([128, 512], F32, tag="m", name=f"pv{t2}")
                    for lc in range(4):
                        nc.tensor.matmul(
                            ps[:],
                            lhsT=kv_t[:, lc, t2 * 128:(t2 + 1) * 128],
                            rhs=wv_sb[:, lc, :],
                            start=(lc == 0), stop=(lc == 3),
                        )
                    nc.vector.tensor_copy(vG[:, kc, :], ps[:])
                # q_c per head
                for h in range(GH):
                    ps = PSM.tile([128, 512], F32, tag="m", name=f"pqc{h}")
                    for lc in range(4):
                        nc.tensor.matmul(
                            ps[0:C, :],
                            lhsT=wqc_sb[:, lc, h * C:(h + 1) * C],
                            rhs=q_t[:, lc, :],
                            start=(lc == 0), stop=(lc == 3),
                        )
                    nc.vector.tensor_copy(qT[0:C, h, tsl], ps[0:C, :])
                # q_rope: stacked 4h x 32 rope + rot; combine in place
                psr = PSM.tile([128, 512], F32, tag="m", name="pqr")
                psrr = PSM.tile([128, 512], F32, tag="m", name="pqrr")
                for lc in range(4):
                    nc.tensor.matmul(
                        psr[:], lhsT=wqr_sb[:, lc, :],
                        rhs=q_t[:, lc, :],
                        start=(lc == 0), stop=(lc == 3),
                    )
                for lc in range(4):
                    nc.tensor.matmul(
                        psrr[:], lhsT=wqrr_sb[:, lc, :],
                        rhs=q_t[:, lc, :],
                        start=(lc == 0), stop=(lc == 3),
                    )
                t2b = TMP.tile([128, 512], F32, tag="t2b", name="t2b")
                t3 = TMP.tile([128, 512], BF16, tag="t3b", name="t3b")
                nc.vector.tensor_mul(psr[:], psr[:], cos4[:, tsl])
                nc.vector.tensor_mul(t2b[:], psrr[:], sin4[:, tsl])
                nc.vector.tensor_add(t3[:], psr[:], t2b[:])
                for h in range(GH):
                    nc.sync.dma_start(
                        qT[C:128, h, tsl], t3[32 * h:32 * h + 32, :]
                    )

            def phC(c):
                """attention for query chunk c (keys 0..4c+3), all 4 heads."""
                csl = slice(c * 512, (c + 1) * 512)
                nkc = 4 * (c + 1)
                kcs = [4 * c + d for d in range(4)] + list(range(4 * c))
                for h in range(GH):
                    ets = {}
                    stack = []  # binary-counter tree: list of (level, tile)

                    def emit_score(kc, i):
                        ps = PSS.tile([128, 512], F32, tag="s",
                                      name=f"ps{i % 2}")
                        nc.tensor.matmul(
                            ps[:],
                            lhsT=kT[:, h, kc * 128:(kc + 1) * 128],
                            rhs=qT[:, h, csl],
                            start=True, stop=True,
                        )
                        et = ETS.tile([128, 512], BF16, tag="e",
                                      name=f"et{i % 6}")
                        nc.scalar.activation(
                            et[:], ps[:], mybir.ActivationFunctionType.Exp
                        )
                        off = kc - 4 * c
                        if off >= 0:
                            nc.vector.tensor_mul(
                                et[:], et[:],
                                mask4[:, off * 512:(off + 1) * 512],
                            )
                        ets[kc] = et

                    pctx = PSC.tile([128, 512], F32, tag="c", name="pctx")

                    def emit_ctx(kc, i):
                        nc.tensor.matmul(
                            pctx[:],
                            lhsT=vG[:, kc, h * DH:(h + 1) * DH],
                            rhs=ets[kc][:],
                            start=(i == 0), stop=(i == nkc - 1),
                        )
                        # fold into the tree-sum (DVE, bf16)
                        carry = ets[kc]
                        lvl = 0
                        while stack and stack[-1][0] == lvl:
                            _, other = stack.pop()
                            dst = ACC.tile([128, 512], BF16, tag="a",
                                           name=f"acc{i % 5}")
                            nc.vector.tensor_add(dst[:], other[:], carry[:])
                            carry = dst
                            lvl += 1
                        stack.append((lvl, carry))

                    LAG = 3
                    for i, kc in enumerate(kcs):
                        emit_score(kc, i)
                        if i >= LAG:
                            emit_ctx(kcs[i - LAG], i - LAG)
                    for i in range(max(0, nkc - LAG), nkc):
                        emit_ctx(kcs[i], i)
                    # fold remaining tree levels
                    while len(stack) > 1:
                        l1, a = stack.pop()
                        l2, b = stack.pop()
                        dst = ACC.tile([128, 512], BF16, tag="a",
                                       name="accf")
                        nc.vector.tensor_add(dst[:], a[:], b[:])
                        stack.append((max(l1, l2) + 1, dst))
                    # row-sum over keys via all-ones matmul, then normalize
                    prs = PSM.tile([128, 512], F32, tag="m", name="prs")
                    nc.tensor.matmul(
                        prs[:], lhsT=onesb[:], rhs=stack[0][1][:],
                        start=True, stop=True,
                    )
                    rc = RCP.tile([128, 512], F32, tag="rc", name="rc")
                    nc.vector.reciprocal_approx_fast(out=rc[:], in_=prs[:])
                    nc.vector.tensor_mul(ctxT[:, h, csl], pctx[:], rc[:])

            def phD(c):
                """partial out-projection for query chunk c (4 q-blocks)."""
                if WO[0] is None:
                    WO[0] = tc.alloc_tile_pool(name="wo", bufs=1, side="right")
                    wo_sb[0] = WO[0].tile([128, 4, HID], BF16, tag="wo",
                                          name="wo")
                    for hc in range(4):
                        nc.sync.dma_start(wo_sb[0][:, hc, :], woT_r[hc])
                for qb in range(c * 4, c * 4 + 4):
                    for oc in range(4):
                        ps = PSM.tile([128, 512], F32, tag="m",
                                      name=f"po{oc % 3}")
                        for h in range(GH):
                            nc.tensor.matmul(
                                ps[:],
                                lhsT=ctxT[:, h, qb * 128:(qb + 1) * 128],
                                rhs=wo_sb[0][:, h, oc * 512:(oc + 1) * 512],
                                start=(h == 0), stop=(h == 3),
                            )
                        ot = OT.tile([128, 512], F32, tag="ot", name="ot")
                        nc.vector.tensor_copy(ot[:], ps[:])
                        nc.sync.dma_start(
                            out_d[qb * 128:(qb + 1) * 128,
                                  oc * 512:(oc + 1) * 512],
                            ot[:],
                        )

            # ---------------- master schedule ----------------
            lat0 = phA(0)
            phB(0, *lat0)
            lat1 = phA(1)
            phC(0)
            phB(1, *lat1)
            lat2 = phA(2)
            phC(1)
            phB(2, *lat2)
            lat3 = phA(3)
            WD.release()
            phD(0)
            phC(2)
            phB(3, *lat3)
            phD(1)
            phC(3)
            phD(2)
            phD(3)
            if WO[0] is not None:
                WO[0].release()

    nc.compile()
    return nc


def _rot_rows(w):
    # rows of w are the rope dim; rot(w) @ lat == rotate_half(w @ lat)
    hR = w.shape[0] // 2
    return np.concatenate([-w[hR:], w[:hR]], axis=0)


def _prep_inputs(inputs):
    x = np.asarray(inputs["x"], np.float32)
    Wq_down = np.asarray(inputs["Wq_down"], np.float32)
    Wq_up = np.asarray(inputs["Wq_up"], np.float32)
    Wq_rope = np.asarray(inputs["Wq_rope"], np.float32)
    Wkv_down = np.asarray(inputs["Wkv_down"], np.float32)
    Wk_up = np.asarray(inputs["Wk_up"], np.float32)
    Wk_rope = np.asarray(inputs["Wk_rope"], np.float32)
    Wv_up = np.asarray(inputs["Wv_up"], np.float32)
    Wo = np.asarray(inputs["Wo"], np.float32)

    s = np.float32(1.0 / np.sqrt(DH))

    wd_kvT = np.ascontiguousarray(Wkv_down.T).astype(BF16NP)
    wd_qT = np.ascontiguousarray(Wq_down.T).astype(BF16NP)
    wkr2 = np.concatenate([Wk_rope, _rot_rows(Wk_rope)], axis=0)  # [64, HID]
    wkr2T = np.ascontiguousarray(wkr2.T).astype(BF16NP)

    inv_freq = (1.0 / (10000.0 ** (np.arange(0, R, 2, dtype=np.float32) / R)))
    t = np.arange(S, dtype=np.float32)
    freqs = t[:, None] * inv_freq[None, :]
    emb = np.concatenate([freqs, freqs], axis=-1)          # [S, R]
    cos4 = np.tile(np.cos(emb).T, (4, 1)).astype(BF16NP)   # [128, S]
    sin4 = np.tile(np.sin(emb).T, (4, 1)).astype(BF16NP)

    kar = np.arange(128)[:, None]
    qar = np.arange(512)[None, :]
    mask4 = np.empty((128, NQC * 512), np.float32)
    for off in range(4):
        mask4[:, off * 512:(off + 1) * 512] = (
            (128 * off + kar) <= qar
        ).astype(np.float32)
    mask4 = mask4.astype(BF16NP)

    per_g = []
    for g in range(4):
        hsl = slice(g * GH, (g + 1) * GH)
        wk_p = np.concatenate(
            [Wk_up[h * C:(h + 1) * C] for h in range(g * GH, (g + 1) * GH)],
            axis=0)                                         # [384, LAT]
        wv_p = np.concatenate(
            [Wv_up[h * DH:(h + 1) * DH] for h in range(g * GH, (g + 1) * GH)],
            axis=0)                                         # [512, LAT]
        wqc_p = np.concatenate(
            [Wq_up[h * C:(h + 1) * C] for h in range(g * GH, (g + 1) * GH)],
            axis=0) * s
        wqr_p = np.concatenate(
            [Wq_rope[h * R:(h + 1) * R] for h in range(g * GH, (g + 1) * GH)],
            axis=0) * s
        wqrr_p = np.concatenate(
            [_rot_rows(Wq_rope[h * R:(h + 1) * R])
             for h in range(g * GH, (g + 1) * GH)], axis=0) * s
        wo_g = Wo[:, g * GH * DH:(g + 1) * GH * DH]         # [HID, 512]
        per_g.append({
            "wk_pT": np.ascontiguousarray(wk_p.T).astype(BF16NP),
            "wv_pT": np.ascontiguousarray(wv_p.T).astype(BF16NP),
            "wqc_pT": np.ascontiguousarray(wqc_p.T).astype(BF16NP),
            "wqr_pT": np.ascontiguousarray(wqr_p.T).astype(BF16NP),
            "wqrr_pT": np.ascontiguousarray(wqrr_p.T).astype(BF16NP),
            "woT": np.ascontiguousarray(wo_g.T).astype(BF16NP),
        })

    in_maps = []
    for cid in range(8):
        b, g = divmod(cid, 4)
        m = {
            "xbT": np.ascontiguousarray(x[b].T).astype(BF16NP),
            "wd_kvT": wd_kvT, "wd_qT": wd_qT, "wkr2T": wkr2T,
            "cos4": cos4, "sin4": sin4, "mask4": mask4,
        }
        m.update(per_g[g])
        in_maps.append(m)
    return in_maps


_NC_CACHE = None


def kernel(**inputs):
    global _NC_CACHE
    if _NC_CACHE is None:
        _NC_CACHE = build_nc()
    nc = _NC_CACHE
    in_maps = _prep_inputs(inputs)
    res = run_bass_kernel_spmd(nc, in_maps, list(range(8)))
    bo = np.asarray(inputs["bo"], np.float32)
    out = np.empty((B, S, HID), np.float32)
    for b in range(B):
        acc = res.results[4 * b]["out"].astype(np.float32)
        for g in range(1, 4):
            acc = acc + res.results[4 * b + g]["out"]
        out[b] = acc + bo
    return out


# revision 22
# speedup vs baseline: 1.4935x; 1.1397x over previous
"""MLA (multi-head latent attention) Trainium2 kernel, SPMD over 8 NeuronCores.

Sharding: core c = 4*b + g handles batch b and head group g (4 heads),
ALL 2048 query rows.  Causality: query chunk c (512 rows) only attends
key chunks 0..4c+3 (lower triangle), so every core does the same
triangular work -- perfectly balanced, no masks off the diagonal.
Each core emits a PARTIAL out-projection (contraction over its 4 heads'
128-dims); the host sums the 4 partials per batch (+bias).  No
collectives.

On-chip layouts are transposed ([feature, token]) so every matmul
contracts over the partition dim with no on-chip transposes.
rotate_half is folded into host-permuted weight copies; 1/sqrt(dh) into
the q weights; softmax skips the max-pass (scores bounded) and gets its
row-sum from an all-ones matmul over a DVE tree-sum of the exp tiles.
Diagonal score tiles are masked multiplicatively (0/1 bf16) after exp.
"""

import os
import sys
import types

for _p in ("/opt/trn_rl_repo", "/root/.axon_site/_ro/trn_rl_repo"):
    if os.path.isdir(_p) and _p not in sys.path:
        sys.path.append(_p)

import numpy as np
import ml_dtypes

import concourse.bass as bass
import concourse.bacc as bacc_mod
import concourse.mybir as mybir
from concourse.tile import TileContext
from concourse.vector_clock import ScopedClock
from concourse.bass_utils import run_bass_kernel_spmd

F32 = mybir.dt.float32
BF16 = mybir.dt.bfloat16
BF16NP = ml_dtypes.bfloat16

HID, H, LAT, R, DH, C = 2048, 16, 512, 32, 128, 96
B, S = 2, 2048
GH = 4            # heads per core
NQC = 4           # query chunks of 512
NKC = 16          # key chunks of 128


def _patch_tile_drain():
    """The staged walrus rejects a Drain carrying >1 sync-wait. Move the
    TileContext tail-drain waits onto single-wait SP nops."""

    def _drain_and_barrier(self, tick_clock, wait_clock):
        drain_inst = self.nc.sync.drain()
        wait_clock.add_sem_waits(
            drain_inst.ins, ScopedClock({None: tick_clock.global_clock})
        )
        si = drain_inst.ins.sync_info
        if si is not None and len(si.on_wait) > 1:
            waits = list(si.on_wait)
            drain_inst.ins.sync_info = mybir.SyncInfo(
                on_wait=[], on_update=list(si.on_update)
            )
            for w in waits:
                nop = self.nc.sync.nop(nofuse=True)
                nop.ins.sync_info = mybir.SyncInfo(on_wait=[w], on_update=[])
        self.nc.all_engine_barrier()
        assert self.sems is not None
        popped = self.nc._tile_sem_poison_stack.pop()
        assert popped is self._sem_poison
        self.nc.clear_and_free_semaphores(list(self.sems.allocated().values()))
        self.nc.all_engine_barrier()

    TileContext._drain_and_barrier = _drain_and_barrier


def _install_ntff_hook():
    """antenv.axon_hooks is absent in this image; inject it and register the
    ctypes NTFF hook so trace=True / BASS_TRACE can profile."""
    try:
        import antenv

        if "antenv.axon_hooks" not in sys.modules:
            mod = types.ModuleType("antenv.axon_hooks")
            mod._hook = None

            def set_axon_ntff_profile_hook(h):
                mod._hook = h

            def get_axon_ntff_profile_hook():
                return mod._hook

            mod.set_axon_ntff_profile_hook = set_axon_ntff_profile_hook
            mod.get_axon_ntff_profile_hook = get_axon_ntff_profile_hook
            sys.modules["antenv.axon_hooks"] = mod
            antenv.axon_hooks = mod
        boot_dir = "/root/.axon_site/trn_agent_boot"
        so_path = "/opt/axon/libaxon_pjrt.so"
        if os.path.isdir(boot_dir) and os.path.exists(so_path):
            if boot_dir not in sys.path:
                sys.path.append(boot_dir)
            from trn_boot import _ntff_profile_via_ctypes

            hook = _ntff_profile_via_ctypes(so_path)
            if hook is not None:
                sys.modules["antenv.axon_hooks"].set_axon_ntff_profile_hook(hook)
    except Exception:
        pass


_patch_tile_drain()
_install_ntff_hook()


def _dram(nc, name, shape, dtype=F32, out=False):
    return nc.declare_dram_parameter(name, list(shape), dtype, isOutput=out)


def build_nc():
    nc = bacc_mod.Bacc("TRN2")

    xbT = _dram(nc, "xbT", [HID, S], BF16)            # x[b].T
    wd_kvT = _dram(nc, "wd_kvT", [HID, LAT], BF16)    # Wkv_down.T
    wd_qT = _dram(nc, "wd_qT", [HID, LAT], BF16)      # Wq_down.T
    wkr2T = _dram(nc, "wkr2T", [HID, 2 * R], BF16)    # [Wk_rope; rot].T
    wk_pT = _dram(nc, "wk_pT", [LAT, GH * C], BF16)   # 4-head k_c pack .T
    wv_pT = _dram(nc, "wv_pT", [LAT, GH * DH], BF16)  # 4-head v pack .T
    wqc_pT = _dram(nc, "wqc_pT", [LAT, GH * C], BF16)   # 4-head q_c pack /sqrt
    wqr_pT = _dram(nc, "wqr_pT", [LAT, GH * R], BF16)   # 4-head q_rope /sqrt
    wqrr_pT = _dram(nc, "wqrr_pT", [LAT, GH * R], BF16)  # rotated rope /sqrt
    woT = _dram(nc, "woT", [GH * DH, HID], BF16)      # Wo cols for our heads
    cos4_d = _dram(nc, "cos4", [128, S], BF16)        # cos.T tiled 4x
    sin4_d = _dram(nc, "sin4", [128, S], BF16)
    mask4_d = _dram(nc, "mask4", [128, NQC * 512], BF16)  # 0/1 diag masks
    out_d = _dram(nc, "out", [S, HID], out=True)      # partial (4-head) proj

    xbT_r = xbT[:, :].rearrange("(c p two) t -> c p two t", p=128, two=2)
    wd_kvT_r = wd_kvT[:, :].rearrange("(c p two) l -> c p two l", p=128, two=2)
    wd_qT_r = wd_qT[:, :].rearrange("(c p two) l -> c p two l", p=128, two=2)
    wkr2T_r = wkr2T[:, :].rearrange("(c p two) r -> c p two r", p=128, two=2)
    wk_pT_r = wk_pT[:, :].rearrange("(lc p) d -> lc p d", p=128)
    wv_pT_r = wv_pT[:, :].rearrange("(lc p) d -> lc p d", p=128)
    wqc_pT_r = wqc_pT[:, :].rearrange("(lc p) d -> lc p d", p=128)
    wqr_pT_r = wqr_pT[:, :].rearrange("(lc p) d -> lc p d", p=128)
    wqrr_pT_r = wqrr_pT[:, :].rearrange("(lc p) d -> lc p d", p=128)
    woT_r = woT[:, :].rearrange("(hc p) o -> hc p o", p=128)

    with TileContext(nc) as tc:
        with tc.tile_pool(name="perB", bufs=1) as perB, \
             tc.tile_pool(name="lat", bufs=2) as LATP, \
             tc.tile_pool(name="xs", bufs=1) as XS, \
             tc.tile_pool(name="ets", bufs=6) as ETS, \
             tc.tile_pool(name="acc", bufs=8) as ACC, \
             tc.tile_pool(name="rcp", bufs=2) as RCP, \
             tc.tile_pool(name="tmp", bufs=2) as TMP, \
             tc.tile_pool(name="ot", bufs=3) as OT, \
             tc.tile_pool(name="ps_g", bufs=2, space="PSUM") as PSG, \
             tc.tile_pool(name="ps_m", bufs=2, space="PSUM") as PSM, \
             tc.tile_pool(name="ps_s", bufs=2, space="PSUM") as PSS, \
             tc.tile_pool(name="ps_c", bufs=2, space="PSUM") as PSC:

            # ---------- persistent SBUF ----------
            krT = perB.tile([32, S], BF16, tag="krT", name="krT")
            kT = perB.tile([128, GH, S], BF16, tag="kT", name="kT")
            vG = perB.tile([128, NKC, GH * DH], BF16, tag="vG", name="vG")
            qT = perB.tile([128, GH, S], BF16, tag="qT", name="qT")
            ctxT = perB.tile([128, GH, S], BF16, tag="ctxT", name="ctxT")
            cos4 = perB.tile([128, S], BF16, tag="cos4", name="cos4")
            sin4 = perB.tile([128, S], BF16, tag="sin4", name="sin4")
            mask4 = perB.tile([128, NQC * 512], BF16, tag="mask4", name="mask4")
            onesb = perB.tile([128, 128], BF16, tag="ones", name="ones")
            wk_sb = perB.tile([128, 4, GH * C], BF16, tag="wk", name="wk")
            wv_sb = perB.tile([128, 4, GH * DH], BF16, tag="wv", name="wv")
            wqc_sb = perB.tile([128, 4, GH * C], BF16, tag="wqc", name="wqc")
            wqr_sb = perB.tile([128, 4, GH * R], BF16, tag="wqr", name="wqr")
            wqrr_sb = perB.tile([128, 4, GH * R], BF16, tag="wqrr", name="wqrr")

            # down-proj weights: released after phase A(3), wo loaded after.
            # Per-hc tiles so the first matmuls wait only on their own slice;
            # kv weights first (the very first accumulation pass).
            WD = tc.alloc_tile_pool(name="wd", bufs=1, side="right")
            wdkv = [WD.tile([128, 2, LAT], BF16, tag=f"wdkv{hc}",
                            name=f"wdkv{hc}") for hc in range(8)]
            wdq = [WD.tile([128, 2, LAT], BF16, tag=f"wdq{hc}",
                           name=f"wdq{hc}") for hc in range(8)]
            wkr = [WD.tile([128, 2, 2 * R], BF16, tag=f"wkr{hc}",
                           name=f"wkr{hc}") for hc in range(8)]

            def load_wd():
                for hc in range(8):
                    nc.sync.dma_start(wdkv[hc][:], wd_kvT_r[hc])
                for hc in range(8):
                    nc.sync.dma_start(wkr[hc][:], wkr2T_r[hc])
                    nc.sync.dma_start(wdq[hc][:], wd_qT_r[hc])

            def load_x(tq):
                tsl = slice(tq * 512, (tq + 1) * 512)
                xt = [XS.tile([128, 2, 512], BF16, tag=f"xf{hc}",
                              name=f"xf{hc}") for hc in range(8)]
                for hc in range(8):
                    nc.sync.dma_start(xt[hc][:], xbT_r[hc][:, :, tsl])
                return xt

            WO = [None]  # box for the late wo pool
            wo_sb = [None]

            def load_cossin():
                nc.sync.dma_start(cos4[:], cos4_d[:, :])
                nc.sync.dma_start(sin4[:], sin4_d[:, :])

            def load_aux_weights():
                nc.sync.dma_start(mask4[:], mask4_d[:, :])
                nc.gpsimd.memset(onesb[:], 1.0)
                for lc in range(4):
                    nc.sync.dma_start(wk_sb[:, lc, :], wk_pT_r[lc])
                    nc.sync.dma_start(wv_sb[:, lc, :], wv_pT_r[lc])
                    nc.sync.dma_start(wqc_sb[:, lc, :], wqc_pT_r[lc])
                    nc.sync.dma_start(wqr_sb[:, lc, :], wqr_pT_r[lc])
                    nc.sync.dma_start(wqrr_sb[:, lc, :], wqrr_pT_r[lc])

            # ---------------- phase emitters ----------------
            def phA(tq, xt=None):
                """latents for token quarter tq: kv_lat, roped k_rope, q_lat.
                Returns the per-quarter latent tiles for phB(tq)."""
                tsl = slice(tq * 512, (tq + 1) * 512)
                if xt is None:
                    xt = load_x(tq)
                kv_t = LATP.tile([128, 4, 512], BF16, tag="kvlat",
                                 name="kvlat")
                q_t = LATP.tile([128, 4, 512], BF16, tag="qlat", name="qlat")

                # kv_lat: 4 lc passes, 2 rotating psum banks
                for lc in range(4):
                    ps = PSG.tile([128, 512], F32, tag="g", name=f"pkv{lc}")
                    for hc in range(8):
                        for two in range(2):
                            nc.tensor.matmul(
                                ps[:],
                                lhsT=wdkv[hc][:, two, lc * 128:(lc + 1) * 128],
                                rhs=xt[hc][:, two, :],
                                start=(hc == 0 and two == 0),
                                stop=(hc == 7 and two == 1),
                            )
                    nc.vector.tensor_copy(kv_t[:, lc, :], ps[:])
                # k_rope pass (64 rows: [rope; rot]); combine in place
                pkr = PSG.tile([64, 512], F32, tag="g", name="pkr")
                for hc in range(8):
                    for two in range(2):
                        nc.tensor.matmul(
                            pkr[:],
                            lhsT=wkr[hc][:, two, :],
                            rhs=xt[hc][:, two, :],
                            start=(hc == 0 and two == 0),
                            stop=(hc == 7 and two == 1),
                        )
                nc.vector.tensor_mul(pkr[0:32, :], pkr[0:32, :],
                                     cos4[0:32, tsl])
                tkr = TMP.tile([32, 512], F32, tag="tkr", name="tkr")
                nc.vector.tensor_mul(tkr[:], pkr[32:64, :], sin4[0:32, tsl])
                nc.vector.tensor_add(krT[:, tsl], pkr[0:32, :], tkr[:])
                # q_lat: 4 lc passes
                for lc in range(4):
                    ps = PSG.tile([128, 512], F32, tag="g", name=f"pq{lc}")
                    for hc in range(8):
                        for two in range(2):
                            nc.tensor.matmul(
                                ps[:],
                                lhsT=wdq[hc][:, two, lc * 128:(lc + 1) * 128],
                                rhs=xt[hc][:, two, :],
                                start=(hc == 0 and two == 0),
                                stop=(hc == 7 and two == 1),
                            )
                    nc.vector.tensor_copy(q_t[:, lc, :], ps[:])
                return kv_t, q_t

            def phB(tq, kv_t, q_t):
                """per-head projections for quarter tq: k_c, v, q_c, q_rope."""
                tsl = slice(tq * 512, (tq + 1) * 512)
                # k_c per head (96 content rows)
                for h in range(GH):
                    ps = PSM.tile([128, 512], F32, tag="m", name=f"pk{h}")
                    for lc in range(4):
                        nc.tensor.matmul(
                            ps[0:C, :],
                            lhsT=wk_sb[:, lc, h * C:(h + 1) * C],
                            rhs=kv_t[:, lc, :],
                            start=(lc == 0), stop=(lc == 3),
                        )
                    nc.vector.tensor_copy(kT[0:C, h, tsl], ps[0:C, :])
                # shared roped k_rope rows -> kT[96:128] per head (DMA)
                for h in range(GH):
                    nc.sync.dma_start(kT[C:128, h, tsl], krT[:, tsl])
                # v: 4 token sub-chunks of 128, out = [t, 4h*128]
                for t2 in range(4):
                    kc = tq * 4 + t2
                    ps = PSM.tile([128, 512], F32, tag="m", name=f"pv{t2}")
                    for lc in range(4):
                        nc.tensor.matmul(
                            ps[:],
                            lhsT=kv_t[:, lc, t2 * 128:(t2 + 1) * 128],
                            rhs=wv_sb[:, lc, :],
                            start=(lc == 0), stop=(lc == 3),
                        )
                    nc.vector.tensor_copy(vG[:, kc, :], ps[:])
                # q_c per head
                for h in range(GH):
                    ps = PSM.tile([128, 512], F32, tag="m", name=f"pqc{h}")
                    for lc in range(4):
                        nc.tensor.matmul(
                            ps[0:C, :],
                            lhsT=wqc_sb[:, lc, h * C:(h + 1) * C],
                            rhs=q_t[:, lc, :],
                            start=(lc == 0), stop=(lc == 3),
                        )
                    nc.vector.tensor_copy(qT[0:C, h, tsl], ps[0:C, :])
                # q_rope: stacked 4h x 32 rope + rot; combine in place
                psr = PSM.tile([128, 512], F32, tag="m", name="pqr")
                psrr = PSM.tile([128, 512], F32, tag="m", name="pqrr")
                for lc in range(4):
                    nc.tensor.matmul(
                        psr[:], lhsT=wqr_sb[:, lc, :],
                        rhs=q_t[:, lc, :],
                        start=(lc == 0), stop=(lc == 3),
                    )
                for lc in range(4):
                    nc.tensor.matmul(
                        psrr[:], lhsT=wqrr_sb[:, lc, :],
                        rhs=q_t[:, lc, :],
                        start=(lc == 0), stop=(lc == 3),
                    )
                t2b = TMP.tile([128, 512], F32, tag="t2b", name="t2b")
                t3 = TMP.tile([128, 512], BF16, tag="t3b", name="t3b")
                nc.vector.tensor_mul(psr[:], psr[:], cos4[:, tsl])
                nc.vector.tensor_mul(t2b[:], psrr[:], sin4[:, tsl])
                nc.vector.tensor_add(t3[:], psr[:], t2b[:])
                for h in range(GH):
                    nc.sync.dma_start(
                        qT[C:128, h, tsl], t3[32 * h:32 * h + 32, :]
                    )

            def phC_head(c, h):
                """attention main for (chunk c, head h): scores+exp+ctx+tree.
                Returns state for phC_fin."""
                csl = slice(c * 512, (c + 1) * 512)
                nkc = 4 * (c + 1)
                kcs = [4 * c + d for d in range(4)] + list(range(4 * c))
                ets = {}
                stack = []  # binary-counter tree: list of (level, tile)

                def emit_score(kc, i):
                    ps = PSS.tile([128, 512], F32, tag="s", name=f"ps{i % 2}")
                    nc.tensor.matmul(
                        ps[:],
                        lhsT=kT[:, h, kc * 128:(kc + 1) * 128],
                        rhs=qT[:, h, csl],
                        start=True, stop=True,
                    )
                    et = ETS.tile([128, 512], BF16, tag="e", name=f"et{i % 6}")
                    nc.scalar.activation(
                        et[:], ps[:], mybir.ActivationFunctionType.Exp
                    )
                    off = kc - 4 * c
                    if off >= 0:
                        nc.vector.tensor_mul(
                            et[:], et[:], mask4[:, off * 512:(off + 1) * 512]
                        )
                    ets[kc] = et

                pctx = PSC.tile([128, 512], F32, tag="c", name="pctx")

                def emit_ctx(kc, i):
                    nc.tensor.matmul(
                        pctx[:],
                        lhsT=vG[:, kc, h * DH:(h + 1) * DH],
                        rhs=ets[kc][:],
                        start=(i == 0), stop=(i == nkc - 1),
                    )
                    # fold into the tree-sum (DVE, bf16)
                    carry = ets[kc]
                    lvl = 0
                    while stack and stack[-1][0] == lvl:
                        _, other = stack.pop()
                        dst = ACC.tile([128, 512], BF16, tag="a",
                                       name=f"acc{i % 5}")
                        nc.vector.tensor_add(dst[:], other[:], carry[:])
                        carry = dst
                        lvl += 1
                    stack.append((lvl, carry))

                LAG = 3
                for i, kc in enumerate(kcs):
                    emit_score(kc, i)
                    if i >= LAG:
                        emit_ctx(kcs[i - LAG], i - LAG)
                for i in range(max(0, nkc - LAG), nkc):
                    emit_ctx(kcs[i], i)
                # fold remaining tree levels
                while len(stack) > 1:
                    l1, a = stack.pop()
                    l2, b = stack.pop()
                    dst = ACC.tile([128, 512], BF16, tag="a", name="accf")
                    nc.vector.tensor_add(dst[:], a[:], b[:])
                    stack.append((max(l1, l2) + 1, dst))
                return pctx, stack[0][1]

            def phC_fin(c, h, pctx, tsum):
                """row-sum via all-ones matmul, reciprocal, ctx normalize."""
                csl = slice(c * 512, (c + 1) * 512)
                prs = PSM.tile([128, 512], F32, tag="m", name="prs")
                nc.tensor.matmul(
                    prs[:], lhsT=onesb[:], rhs=tsum[:], start=True, stop=True
                )
                rc = RCP.tile([128, 512], F32, tag="rc", name="rc")
                nc.vector.reciprocal_approx_fast(out=rc[:], in_=prs[:])
                nc.vector.tensor_mul(ctxT[:, h, csl], pctx[:], rc[:])

            def phC(c):
                for h in range(GH):
                    pctx, tsum = phC_head(c, h)
                    phC_fin(c, h, pctx, tsum)

            def load_wo():
                if WO[0] is None:
                    WO[0] = tc.alloc_tile_pool(name="wo", bufs=1, side="right")
                    wo_sb[0] = WO[0].tile([128, 4, HID], BF16, tag="wo",
                                          name="wo")
                    for hc in range(4):
                        nc.sync.dma_start(wo_sb[0][:, hc, :], woT_r[hc])

            def phD_qb(qb):
                """partial out-projection for one 128-row query block."""
                for oc in range(4):
                    ps = PSM.tile([128, 512], F32, tag="m",
                                  name=f"po{oc % 3}")
                    for h in range(GH):
                        nc.tensor.matmul(
                            ps[:],
                            lhsT=ctxT[:, h, qb * 128:(qb + 1) * 128],
                            rhs=wo_sb[0][:, h, oc * 512:(oc + 1) * 512],
                            start=(h == 0), stop=(h == 3),
                        )
                    ot = OT.tile([128, 512], F32, tag="ot", name="ot")
                    nc.vector.tensor_copy(ot[:], ps[:])
                    nc.sync.dma_start(
                        out_d[qb * 128:(qb + 1) * 128,
                              oc * 512:(oc + 1) * 512],
                        ot[:],
                    )

            # ---------------- master schedule ----------------
            xt0 = load_x(0)
            load_wd()
            load_cossin()
            lat0 = phA(0, xt0)
            load_aux_weights()
            phB(0, *lat0)
            lat1 = phA(1)
            phC(0)
            phB(1, *lat1)
            lat2 = phA(2)
            phC(1)
            phB(2, *lat2)
            lat3 = phA(3)
            WD.release()
            load_wo()
            # interleave out-proj q-blocks into the attention blocks so the
            # PE has exp-independent work to absorb the ACT (exp) lag
            st = phC_head(2, 0)
            phD_qb(0)
            phC_fin(2, 0, *st)
            st = phC_head(2, 1)
            phD_qb(1)
            phC_fin(2, 1, *st)
            phB(3, *lat3)
            st = phC_head(2, 2)
            phD_qb(2)
            phC_fin(2, 2, *st)
            st = phC_head(2, 3)
            phD_qb(3)
            phC_fin(2, 3, *st)
            for h in range(GH):
                st = phC_head(3, h)
                phD_qb(4 + h)
                phC_fin(3, h, *st)
            for qb in range(8, 16):
                phD_qb(qb)
            if WO[0] is not None:
                WO[0].release()

    nc.compile()
    return nc


def _rot_rows(w):
    # rows of w are the rope dim; rot(w) @ lat == rotate_half(w @ lat)
    hR = w.shape[0] // 2
    return np.concatenate([-w[hR:], w[:hR]], axis=0)


def _prep_inputs(inputs):
    x = np.asarray(inputs["x"], np.float32)
    Wq_down = np.asarray(inputs["Wq_down"], np.float32)
    Wq_up = np.asarray(inputs["Wq_up"], np.float32)
    Wq_rope = np.asarray(inputs["Wq_rope"], np.float32)
    Wkv_down = np.asarray(inputs["Wkv_down"], np.float32)
    Wk_up = np.asarray(inputs["Wk_up"], np.float32)
    Wk_rope = np.asarray(inputs["Wk_rope"], np.float32)
    Wv_up = np.asarray(inputs["Wv_up"], np.float32)
    Wo = np.asarray(inputs["Wo"], np.float32)

    s = np.float32(1.0 / np.sqrt(DH))

    wd_kvT = np.ascontiguousarray(Wkv_down.T).astype(BF16NP)
    wd_qT = np.ascontiguousarray(Wq_down.T).astype(BF16NP)
    wkr2 = np.concatenate([Wk_rope, _rot_rows(Wk_rope)], axis=0)  # [64, HID]
    wkr2T = np.ascontiguousarray(wkr2.T).astype(BF16NP)

    inv_freq = (1.0 / (10000.0 ** (np.arange(0, R, 2, dtype=np.float32) / R)))
    t = np.arange(S, dtype=np.float32)
    freqs = t[:, None] * inv_freq[None, :]
    emb = np.concatenate([freqs, freqs], axis=-1)          # [S, R]
    cos4 = np.tile(np.cos(emb).T, (4, 1)).astype(BF16NP)   # [128, S]
    sin4 = np.tile(np.sin(emb).T, (4, 1)).astype(BF16NP)

    kar = np.arange(128)[:, None]
    qar = np.arange(512)[None, :]
    mask4 = np.empty((128, NQC * 512), np.float32)
    for off in range(4):
        mask4[:, off * 512:(off + 1) * 512] = (
            (128 * off + kar) <= qar
        ).astype(np.float32)
    mask4 = mask4.astype(BF16NP)

    per_g = []
    for g in range(4):
        hsl = slice(g * GH, (g + 1) * GH)
        wk_p = np.concatenate(
            [Wk_up[h * C:(h + 1) * C] for h in range(g * GH, (g + 1) * GH)],
            axis=0)                                         # [384, LAT]
        wv_p = np.concatenate(
            [Wv_up[h * DH:(h + 1) * DH] for h in range(g * GH, (g + 1) * GH)],
            axis=0)                                         # [512, LAT]
        wqc_p = np.concatenate(
            [Wq_up[h * C:(h + 1) * C] for h in range(g * GH, (g + 1) * GH)],
            axis=0) * s
        wqr_p = np.concatenate(
            [Wq_rope[h * R:(h + 1) * R] for h in range(g * GH, (g + 1) * GH)],
            axis=0) * s
        wqrr_p = np.concatenate(
            [_rot_rows(Wq_rope[h * R:(h + 1) * R])
             for h in range(g * GH, (g + 1) * GH)], axis=0) * s
        wo_g = Wo[:, g * GH * DH:(g + 1) * GH * DH]         # [HID, 512]
        per_g.append({
            "wk_pT": np.ascontiguousarray(wk_p.T).astype(BF16NP),
            "wv_pT": np.ascontiguousarray(wv_p.T).astype(BF16NP),
            "wqc_pT": np.ascontiguousarray(wqc_p.T).astype(BF16NP),
            "wqr_pT": np.ascontiguousarray(wqr_p.T).astype(BF16NP),
            "wqrr_pT": np.ascontiguousarray(wqrr_p.T).astype(BF16NP),
            "woT": np.ascontiguousarray(wo_g.T).astype(BF16NP),
        })

    in_maps = []
    for cid in range(8):
        b, g = divmod(cid, 4)
        m = {
            "xbT": np.ascontiguousarray(x[b].T).astype(BF16NP),
            "wd_kvT": wd_kvT, "wd_qT": wd_qT, "wkr2T": wkr2T,
            "cos4": cos4, "sin4": sin4, "mask4": mask4,
        }
        m.update(per_g[g])
        in_maps.append(m)
    return in_maps


_NC_CACHE = None


def kernel(**inputs):
    global _NC_CACHE
    if _NC_CACHE is None:
        _NC_CACHE = build_nc()
    nc = _NC_CACHE
    in_maps = _prep_inputs(inputs)
    res = run_bass_kernel_spmd(nc, in_maps, list(range(8)))
    bo = np.asarray(inputs["bo"], np.float32)
    out = np.empty((B, S, HID), np.float32)
    for b in range(B):
        acc = res.results[4 * b]["out"].astype(np.float32)
        for g in range(1, 4):
            acc = acc + res.results[4 * b + g]["out"]
        out[b] = acc + bo
    return out


# revision 41
# speedup vs baseline: 1.5431x; 1.0332x over previous
"""MLA (multi-head latent attention) Trainium2 kernel, SPMD over 8 NeuronCores.

Sharding: core c = 4*b + g handles batch b and head group g (4 heads),
ALL 2048 query rows.  Causality: query chunk c (512 rows) only attends
key chunks 0..4c+3 (lower triangle), so every core does the same
triangular work -- perfectly balanced, no masks off the diagonal.
Each core emits a PARTIAL out-projection (contraction over its 4 heads'
128-dims); the host sums the 4 partials per batch (+bias).  No
collectives.

On-chip layouts are transposed ([feature, token]) so every matmul
contracts over the partition dim with no on-chip transposes.
rotate_half is folded into host-permuted weight copies; 1/sqrt(dh) into
the q weights; softmax skips the max-pass (scores bounded) and gets its
row-sum from an all-ones matmul over a DVE tree-sum of the exp tiles.
Diagonal score tiles are masked multiplicatively (0/1 bf16) after exp.
"""

import os
import sys
import types

for _p in ("/opt/trn_rl_repo", "/root/.axon_site/_ro/trn_rl_repo"):
    if os.path.isdir(_p) and _p not in sys.path:
        sys.path.append(_p)

import numpy as np
import ml_dtypes

import concourse.bass as bass
import concourse.bacc as bacc_mod
import concourse.mybir as mybir
from concourse.tile import TileContext
from concourse.vector_clock import ScopedClock
from concourse.bass_utils import run_bass_kernel_spmd

F32 = mybir.dt.float32
BF16 = mybir.dt.bfloat16
BF16NP = ml_dtypes.bfloat16

HID, H, LAT, R, DH, C = 2048, 16, 512, 32, 128, 96
B, S = 2, 2048
GH = 4            # heads per core
NQC = 4           # query chunks of 512
NKC = 16          # key chunks of 128


def _patch_tile_drain():
    """The staged walrus rejects a Drain carrying >1 sync-wait. Move the
    TileContext tail-drain waits onto single-wait SP nops."""

    def _drain_and_barrier(self, tick_clock, wait_clock):
        drain_inst = self.nc.sync.drain()
        wait_clock.add_sem_waits(
            drain_inst.ins, ScopedClock({None: tick_clock.global_clock})
        )
        si = drain_inst.ins.sync_info
        if si is not None and len(si.on_wait) > 1:
            waits = list(si.on_wait)
            drain_inst.ins.sync_info = mybir.SyncInfo(
                on_wait=[], on_update=list(si.on_update)
            )
            for w in waits:
                nop = self.nc.sync.nop(nofuse=True)
                nop.ins.sync_info = mybir.SyncInfo(on_wait=[w], on_update=[])
        self.nc.all_engine_barrier()
        assert self.sems is not None
        popped = self.nc._tile_sem_poison_stack.pop()
        assert popped is self._sem_poison
        self.nc.clear_and_free_semaphores(list(self.sems.allocated().values()))
        self.nc.all_engine_barrier()

    TileContext._drain_and_barrier = _drain_and_barrier


def _install_ntff_hook():
    """antenv.axon_hooks is absent in this image; inject it and register the
    ctypes NTFF hook so trace=True / BASS_TRACE can profile."""
    try:
        import antenv

        if "antenv.axon_hooks" not in sys.modules:
            mod = types.ModuleType("antenv.axon_hooks")
            mod._hook = None

            def set_axon_ntff_profile_hook(h):
                mod._hook = h

            def get_axon_ntff_profile_hook():
                return mod._hook

            mod.set_axon_ntff_profile_hook = set_axon_ntff_profile_hook
            mod.get_axon_ntff_profile_hook = get_axon_ntff_profile_hook
            sys.modules["antenv.axon_hooks"] = mod
            antenv.axon_hooks = mod
        boot_dir = "/root/.axon_site/trn_agent_boot"
        so_path = "/opt/axon/libaxon_pjrt.so"
        if os.path.isdir(boot_dir) and os.path.exists(so_path):
            if boot_dir not in sys.path:
                sys.path.append(boot_dir)
            from trn_boot import _ntff_profile_via_ctypes

            hook = _ntff_profile_via_ctypes(so_path)
            if hook is not None:
                sys.modules["antenv.axon_hooks"].set_axon_ntff_profile_hook(hook)
    except Exception:
        pass


_patch_tile_drain()
_install_ntff_hook()


def _dram(nc, name, shape, dtype=F32, out=False):
    return nc.declare_dram_parameter(name, list(shape), dtype, isOutput=out)


def build_nc():
    nc = bacc_mod.Bacc("TRN2")

    xqT = _dram(nc, "xqT", [HID, 512], BF16)          # own token-quarter of x[b].T
    cosq_d = _dram(nc, "cosq", [32, 512], BF16)       # cos.T own quarter
    sinq_d = _dram(nc, "sinq", [32, 512], BF16)
    wd_kvT = _dram(nc, "wd_kvT", [HID, LAT], BF16)    # Wkv_down.T
    wd_qT = _dram(nc, "wd_qT", [HID, LAT], BF16)      # Wq_down.T
    wkr2T = _dram(nc, "wkr2T", [HID, 2 * R], BF16)    # [Wk_rope; rot].T
    wk_pT = _dram(nc, "wk_pT", [LAT, GH * C], BF16)   # 4-head k_c pack .T
    wv_pT = _dram(nc, "wv_pT", [LAT, GH * DH], BF16)  # 4-head v pack .T
    wqc_pT = _dram(nc, "wqc_pT", [LAT, GH * C], BF16)   # 4-head q_c pack /sqrt
    wqr_pT = _dram(nc, "wqr_pT", [LAT, GH * R], BF16)   # 4-head q_rope /sqrt
    wqrr_pT = _dram(nc, "wqrr_pT", [LAT, GH * R], BF16)  # rotated rope /sqrt
    woT = _dram(nc, "woT", [GH * DH, HID], BF16)      # Wo cols for our heads
    cos4_d = _dram(nc, "cos4", [128, S], BF16)        # cos.T tiled 4x
    sin4_d = _dram(nc, "sin4", [128, S], BF16)
    mask4_d = _dram(nc, "mask4", [128, NQC * 512], BF16)  # 0/1 diag masks
    out_d = _dram(nc, "out", [S, HID], out=True)      # partial (4-head) proj

    xqT_r = xqT[:, :].rearrange("(c p two) t -> c p two t", p=128, two=2)
    wd_kvT_r = wd_kvT[:, :].rearrange("(c p two) l -> c p two l", p=128, two=2)
    wd_qT_r = wd_qT[:, :].rearrange("(c p two) l -> c p two l", p=128, two=2)
    wkr2T_r = wkr2T[:, :].rearrange("(c p two) r -> c p two r", p=128, two=2)
    wk_pT_r = wk_pT[:, :].rearrange("(lc p) d -> lc p d", p=128)
    wv_pT_r = wv_pT[:, :].rearrange("(lc p) d -> lc p d", p=128)
    wqc_pT_r = wqc_pT[:, :].rearrange("(lc p) d -> lc p d", p=128)
    wqr_pT_r = wqr_pT[:, :].rearrange("(lc p) d -> lc p d", p=128)
    wqrr_pT_r = wqrr_pT[:, :].rearrange("(lc p) d -> lc p d", p=128)
    woT_r = woT[:, :].rearrange("(hc p) o -> hc p o", p=128)

    with TileContext(nc) as tc:
        with tc.tile_pool(name="perB", bufs=1) as perB, \
             tc.tile_pool(name="lat", bufs=2) as LATP, \
             tc.tile_pool(name="lato", bufs=1) as LATO, \
             tc.tile_pool(name="gth", bufs=1, space="DRAM") as GTH, \
             tc.tile_pool(name="xs", bufs=1) as XS, \
             tc.tile_pool(name="ets", bufs=6) as ETS, \
             tc.tile_pool(name="acc", bufs=8) as ACC, \
             tc.tile_pool(name="rcp", bufs=2) as RCP, \
             tc.tile_pool(name="tmp", bufs=2) as TMP, \
             tc.tile_pool(name="ot", bufs=3) as OT, \
             tc.tile_pool(name="ps_g", bufs=2, space="PSUM") as PSG, \
             tc.tile_pool(name="ps_m", bufs=2, space="PSUM") as PSM, \
             tc.tile_pool(name="ps_s", bufs=2, space="PSUM") as PSS, \
             tc.tile_pool(name="ps_c", bufs=2, space="PSUM") as PSC:

            # ---------- persistent SBUF ----------
            cosq = perB.tile([32, 512], BF16, tag="cosq", name="cosq")
            sinq = perB.tile([32, 512], BF16, tag="sinq", name="sinq")
            kT = perB.tile([128, GH, S], BF16, tag="kT", name="kT")
            vG = perB.tile([128, NKC, GH * DH], BF16, tag="vG", name="vG")
            qT = perB.tile([128, GH, S], BF16, tag="qT", name="qT")
            ctxT = perB.tile([128, GH, S], BF16, tag="ctxT", name="ctxT")
            cos4 = perB.tile([128, S], BF16, tag="cos4", name="cos4")
            sin4 = perB.tile([128, S], BF16, tag="sin4", name="sin4")
            mask4 = perB.tile([128, NQC * 512], BF16, tag="mask4", name="mask4")
            onesb = perB.tile([128, 128], BF16, tag="ones", name="ones")
            wk_sb = perB.tile([128, 4, GH * C], BF16, tag="wk", name="wk")
            wv_sb = perB.tile([128, 4, GH * DH], BF16, tag="wv", name="wv")
            wqc_sb = perB.tile([128, 4, GH * C], BF16, tag="wqc", name="wqc")
            wqr_sb = perB.tile([128, 4, GH * R], BF16, tag="wqr", name="wqr")
            wqrr_sb = perB.tile([128, 4, GH * R], BF16, tag="wqrr", name="wqrr")

            # down-proj weights: released after phase A(3), wo loaded after.
            # Per-hc tiles so the first matmuls wait only on their own slice;
            # kv weights first (the very first accumulation pass).
            WD = tc.alloc_tile_pool(name="wd", bufs=1, side="right")
            wdkv = [WD.tile([128, 2, LAT], BF16, tag=f"wdkv{hc}",
                            name=f"wdkv{hc}") for hc in range(8)]
            wdq = [WD.tile([128, 2, LAT], BF16, tag=f"wdq{hc}",
                           name=f"wdq{hc}") for hc in range(8)]
            wkr = [WD.tile([128, 2, 2 * R], BF16, tag=f"wkr{hc}",
                           name=f"wkr{hc}") for hc in range(8)]

            def load_wd():
                for hc in range(8):
                    nc.sync.dma_start(wdkv[hc][:], wd_kvT_r[hc])
                for hc in range(8):
                    nc.sync.dma_start(wkr[hc][:], wkr2T_r[hc])
                    nc.sync.dma_start(wdq[hc][:], wd_qT_r[hc])

            WO = [None]  # box for the late wo pool
            wo_sb = [None]

            # DRAM bounce buffers for the latent all-gathers (4-core groups)
            lat_kv_mine = GTH.tile([128, 2048], BF16, tag="kvm", name="kvm")
            lat_kv_all = GTH.tile([512, 2048], BF16, tag="kva", name="kva")
            lat_q_mine = GTH.tile([128, 2560], BF16, tag="qm", name="qm")
            lat_q_all = GTH.tile([512, 2560], BF16, tag="qa", name="qa")
            RG = [[0, 1, 2, 3], [4, 5, 6, 7]]

            def load_cossin():
                nc.sync.dma_start(cosq[:], cosq_d[:, :])
                nc.sync.dma_start(sinq[:], sinq_d[:, :])

            def load_aux_weights():
                nc.sync.dma_start(cos4[:], cos4_d[:, :])
                nc.sync.dma_start(sin4[:], sin4_d[:, :])
                nc.sync.dma_start(mask4[:], mask4_d[:, :])
                nc.gpsimd.memset(onesb[:], 1.0)
                for lc in range(4):
                    nc.sync.dma_start(wk_sb[:, lc, :], wk_pT_r[lc])
                    nc.sync.dma_start(wv_sb[:, lc, :], wv_pT_r[lc])
                    nc.sync.dma_start(wqc_sb[:, lc, :], wqc_pT_r[lc])
                    nc.sync.dma_start(wqr_sb[:, lc, :], wqr_pT_r[lc])
                    nc.sync.dma_start(wqrr_sb[:, lc, :], wqrr_pT_r[lc])

            # ---------------- phase emitters ----------------
            # each core computes the latents ONLY for its own token quarter
            # (from its xqT input), then the 4-core batch group all-gathers
            # them through DRAM bounce buffers.
            def phA_own_kv(xt):
                """kv_lat for the own quarter -> DRAM -> AllGather."""
                kv_own = LATO.tile([128, 2048], BF16, tag="kvown",
                                   name="kvown")
                for lc in range(4):
                    ps = PSG.tile([128, 512], F32, tag="g", name=f"pkv{lc}")
                    for hc in range(8):
                        for two in range(2):
                            nc.tensor.matmul(
                                ps[:],
                                lhsT=wdkv[hc][:, two, lc * 128:(lc + 1) * 128],
                                rhs=xt[hc][:, two, :],
                                start=(hc == 0 and two == 0),
                                stop=(hc == 7 and two == 1),
                            )
                    nc.vector.tensor_copy(
                        kv_own[:, lc * 512:(lc + 1) * 512], ps[:]
                    )
                nc.sync.dma_start(lat_kv_mine[:, :], kv_own[:])
                nc.gpsimd.collective_compute(
                    "AllGather", mybir.AluOpType.bypass, replica_groups=RG,
                    ins=[lat_kv_mine[:, :].opt()],
                    outs=[lat_kv_all[:, :].opt()],
                )

            def phA_own_q(xt, cosq, sinq):
                """roped k_rope + q_lat for the own quarter -> AllGather."""
                q_own = LATO.tile([128, 2048], BF16, tag="qown", name="qown")
                kr_own = LATO.tile([32, 512], BF16, tag="krown", name="krown")
                pkr = PSG.tile([64, 512], F32, tag="g", name="pkr")
                for hc in range(8):
                    for two in range(2):
                        nc.tensor.matmul(
                            pkr[:],
                            lhsT=wkr[hc][:, two, :],
                            rhs=xt[hc][:, two, :],
                            start=(hc == 0 and two == 0),
                            stop=(hc == 7 and two == 1),
                        )
                nc.vector.tensor_mul(pkr[0:32, :], pkr[0:32, :], cosq[:])
                tkr = TMP.tile([32, 512], F32, tag="tkr", name="tkr")
                nc.vector.tensor_mul(tkr[:], pkr[32:64, :], sinq[:])
                nc.vector.tensor_add(kr_own[:], pkr[0:32, :], tkr[:])
                for lc in range(4):
                    ps = PSG.tile([128, 512], F32, tag="g", name=f"pq{lc}")
                    for hc in range(8):
                        for two in range(2):
                            nc.tensor.matmul(
                                ps[:],
                                lhsT=wdq[hc][:, two, lc * 128:(lc + 1) * 128],
                                rhs=xt[hc][:, two, :],
                                start=(hc == 0 and two == 0),
                                stop=(hc == 7 and two == 1),
                            )
                    nc.vector.tensor_copy(
                        q_own[:, lc * 512:(lc + 1) * 512], ps[:]
                    )
                nc.sync.dma_start(lat_q_mine[:, 0:2048], q_own[:])
                nc.sync.dma_start(lat_q_mine[0:32, 2048:2560], kr_own[:])
                nc.gpsimd.collective_compute(
                    "AllGather", mybir.AluOpType.bypass, replica_groups=RG,
                    ins=[lat_q_mine[:, :].opt()],
                    outs=[lat_q_all[:, :].opt()],
                )

            def gather_in_kv(tq):
                kv_t = LATP.tile([128, 2048], BF16, tag="kvg",
                                 name=f"kv{tq}")
                nc.sync.dma_start(
                    kv_t[:], lat_kv_all[128 * tq:128 * (tq + 1), :]
                )
                return kv_t

            def gather_in_q(tq):
                q_t = LATP.tile([128, 2048], BF16, tag="qg",
                                name=f"q{tq}")
                nc.sync.dma_start(
                    q_t[:], lat_q_all[128 * tq:128 * (tq + 1), 0:2048]
                )
                # roped k_rope rows straight into kT[96:128] per head
                for h in range(GH):
                    nc.sync.dma_start(
                        kT[C:128, h, tq * 512:(tq + 1) * 512],
                        lat_q_all[128 * tq:128 * tq + 32, 2048:2560],
                    )
                return q_t

            def phB_kv(tq, kv_t):
                """k_c + v projections for quarter tq."""
                tsl = slice(tq * 512, (tq + 1) * 512)
                # k_c per head (96 content rows)
                for h in range(GH):
                    ps = PSM.tile([128, 512], F32, tag="m", name=f"pk{h}")
                    for lc in range(4):
                        nc.tensor.matmul(
                            ps[0:C, :],
                            lhsT=wk_sb[:, lc, h * C:(h + 1) * C],
                            rhs=kv_t[:, lc * 512:(lc + 1) * 512],
                            start=(lc == 0), stop=(lc == 3),
                        )
                    nc.vector.tensor_copy(kT[0:C, h, tsl], ps[0:C, :])
                # v: 4 token sub-chunks of 128, out = [t, 4h*128]
                for t2 in range(4):
                    kc = tq * 4 + t2
                    ps = PSM.tile([128, 512], F32, tag="m", name=f"pv{t2}")
                    for lc in range(4):
                        nc.tensor.matmul(
                            ps[:],
                            lhsT=kv_t[:, lc * 512 + t2 * 128:
                                      lc * 512 + (t2 + 1) * 128],
                            rhs=wv_sb[:, lc, :],
                            start=(lc == 0), stop=(lc == 3),
                        )
                    nc.vector.tensor_copy(vG[:, kc, :], ps[:])

            def phB_qc(tq, q_t, heads):
                """q_c projections for the given heads of quarter tq."""
                tsl = slice(tq * 512, (tq + 1) * 512)
                for h in heads:
                    ps = PSM.tile([128, 512], F32, tag="m", name=f"pqc{h}")
                    for lc in range(4):
                        nc.tensor.matmul(
                            ps[0:C, :],
                            lhsT=wqc_sb[:, lc, h * C:(h + 1) * C],
                            rhs=q_t[:, lc * 512:(lc + 1) * 512],
                            start=(lc == 0), stop=(lc == 3),
                        )
                    nc.vector.tensor_copy(qT[0:C, h, tsl], ps[0:C, :])

            def phB_qr(tq, q_t):
                """q_rope: stacked 4h x 32 rope + rot; combine, scatter."""
                tsl = slice(tq * 512, (tq + 1) * 512)
                psr = PSM.tile([128, 512], F32, tag="m", name="pqr")
                psrr = PSM.tile([128, 512], F32, tag="m", name="pqrr")
                for lc in range(4):
                    nc.tensor.matmul(
                        psr[:], lhsT=wqr_sb[:, lc, :],
                        rhs=q_t[:, lc * 512:(lc + 1) * 512],
                        start=(lc == 0), stop=(lc == 3),
                    )
                for lc in range(4):
                    nc.tensor.matmul(
                        psrr[:], lhsT=wqrr_sb[:, lc, :],
                        rhs=q_t[:, lc * 512:(lc + 1) * 512],
                        start=(lc == 0), stop=(lc == 3),
                    )
                t2b = TMP.tile([128, 512], F32, tag="t2b", name="t2b")
                t3 = TMP.tile([128, 512], BF16, tag="t3b", name="t3b")
                nc.vector.tensor_mul(psr[:], psr[:], cos4[:, tsl])
                nc.vector.tensor_mul(t2b[:], psrr[:], sin4[:, tsl])
                nc.vector.tensor_add(t3[:], psr[:], t2b[:])
                for h in range(GH):
                    nc.sync.dma_start(
                        qT[C:128, h, tsl], t3[32 * h:32 * h + 32, :]
                    )

            def phC_head(c, h):
                """attention main for (chunk c, head h): scores+exp+ctx+tree.
                Returns state for phC_fin."""
                csl = slice(c * 512, (c + 1) * 512)
                nkc = 4 * (c + 1)
                kcs = [4 * c + d for d in range(4)] + list(range(4 * c))
                ets = {}
                stack = []  # binary-counter tree: list of (level, tile)

                def emit_score(kc, i):
                    ps = PSS.tile([128, 512], F32, tag="s", name=f"ps{i % 2}")
                    nc.tensor.matmul(
                        ps[:],
                        lhsT=kT[:, h, kc * 128:(kc + 1) * 128],
                        rhs=qT[:, h, csl],
                        start=True, stop=True,
                    )
                    et = ETS.tile([128, 512], BF16, tag="e", name=f"et{i % 6}")
                    nc.scalar.activation(
                        et[:], ps[:], mybir.ActivationFunctionType.Exp
                    )
                    off = kc - 4 * c
                    if off >= 0:
                        nc.vector.tensor_mul(
                            et[:], et[:], mask4[:, off * 512:(off + 1) * 512]
                        )
                    ets[kc] = et

                pctx = PSC.tile([128, 512], F32, tag="c", name="pctx")

                def emit_ctx(kc, i):
                    nc.tensor.matmul(
                        pctx[:],
                        lhsT=vG[:, kc, h * DH:(h + 1) * DH],
                        rhs=ets[kc][:],
                        start=(i == 0), stop=(i == nkc - 1),
                    )
                    # fold into the tree-sum (DVE, bf16)
                    carry = ets[kc]
                    lvl = 0
                    while stack and stack[-1][0] == lvl:
                        _, other = stack.pop()
                        dst = ACC.tile([128, 512], BF16, tag="a",
                                       name=f"acc{i % 5}")
                        nc.vector.tensor_add(dst[:], other[:], carry[:])
                        carry = dst
                        lvl += 1
                    stack.append((lvl, carry))

                LAG = 3
                for i, kc in enumerate(kcs):
                    emit_score(kc, i)
                    if i >= LAG:
                        emit_ctx(kcs[i - LAG], i - LAG)
                for i in range(max(0, nkc - LAG), nkc):
                    emit_ctx(kcs[i], i)
                # fold remaining tree levels
                while len(stack) > 1:
                    l1, a = stack.pop()
                    l2, b = stack.pop()
                    dst = ACC.tile([128, 512], BF16, tag="a", name="accf")
                    nc.vector.tensor_add(dst[:], a[:], b[:])
                    stack.append((max(l1, l2) + 1, dst))
                return pctx, stack[0][1]

            def phC_fin(c, h, pctx, tsum):
                """row-sum via all-ones matmul, reciprocal, ctx normalize."""
                csl = slice(c * 512, (c + 1) * 512)
                prs = PSM.tile([128, 512], F32, tag="m", name="prs")
                nc.tensor.matmul(
                    prs[:], lhsT=onesb[:], rhs=tsum[:], start=True, stop=True
                )
                rc = RCP.tile([128, 512], F32, tag="rc", name="rc")
                nc.vector.reciprocal_approx_fast(out=rc[:], in_=prs[:])
                nc.vector.tensor_mul(ctxT[:, h, csl], pctx[:], rc[:])

            def phC(c):
                for h in range(GH):
                    pctx, tsum = phC_head(c, h)
                    phC_fin(c, h, pctx, tsum)

            def load_wo():
                if WO[0] is None:
                    WO[0] = tc.alloc_tile_pool(name="wo", bufs=1, side="right")
                    wo_sb[0] = WO[0].tile([128, 4, HID], BF16, tag="wo",
                                          name="wo")
                    for hc in range(4):
                        nc.sync.dma_start(wo_sb[0][:, hc, :], woT_r[hc])

            def phD_qb(qb):
                """partial out-projection for one 128-row query block."""
                for oc in range(4):
                    ps = PSM.tile([128, 512], F32, tag="m",
                                  name=f"po{oc % 3}")
                    for h in range(GH):
                        nc.tensor.matmul(
                            ps[:],
                            lhsT=ctxT[:, h, qb * 128:(qb + 1) * 128],
                            rhs=wo_sb[0][:, h, oc * 512:(oc + 1) * 512],
                            start=(h == 0), stop=(h == 3),
                        )
                    ot = OT.tile([128, 512], F32, tag="ot", name="ot")
                    nc.vector.tensor_copy(ot[:], ps[:])
                    nc.sync.dma_start(
                        out_d[qb * 128:(qb + 1) * 128,
                              oc * 512:(oc + 1) * 512],
                        ot[:],
                    )

            # ---------------- master schedule ----------------
            xt = [XS.tile([128, 2, 512], BF16, tag=f"xf{hc}",
                          name=f"xf{hc}") for hc in range(8)]
            for hc in range(8):
                nc.sync.dma_start(xt[hc][:], xqT_r[hc])
            load_wd()
            load_cossin()
            phA_own_kv(xt)
            phA_own_q(xt, cosq, sinq)
            WD.release()
            load_aux_weights()
            for tq in range(4):
                kv_t = gather_in_kv(tq)
                phB_kv(tq, kv_t)
            q_ts = {}
            for tq in range(2):
                q_ts[tq] = gather_in_q(tq)
                phB_qc(tq, q_ts[tq], range(GH))
                phB_qr(tq, q_ts[tq])

            def start_q(tq):
                q_ts[tq] = gather_in_q(tq)
                phB_qc(tq, q_ts[tq], [0, 1])

            # attention; interleave exp-independent PE work (remaining q
            # projections, then out-proj q-blocks) to absorb the ACT lag
            fillers = [
                lambda: start_q(2),
                lambda: phB_qc(2, q_ts[2], [2, 3]),
                lambda: phB_qr(2, q_ts[2]),
                lambda: start_q(3),
                lambda: (load_wo(), phB_qc(3, q_ts[3], [2, 3])),
                lambda: phB_qr(3, q_ts[3]),
                lambda: phD_qb(0),
                lambda: phD_qb(1),
                lambda: phD_qb(2),
                lambda: phD_qb(3),
                lambda: phD_qb(4),
                lambda: phD_qb(5),
                lambda: phD_qb(6),
                lambda: phD_qb(7),
                lambda: phD_qb(8),
                lambda: phD_qb(9),
            ]
            fi = 0
            for c in range(NQC):
                for h in range(GH):
                    st = phC_head(c, h)
                    fillers[fi]()
                    fi += 1
                    phC_fin(c, h, *st)
            for qb in range(10, 16):
                phD_qb(qb)
            if WO[0] is not None:
                WO[0].release()

    nc.compile()
    return nc


def _rot_rows(w):
    # rows of w are the rope dim; rot(w) @ lat == rotate_half(w @ lat)
    hR = w.shape[0] // 2
    return np.concatenate([-w[hR:], w[:hR]], axis=0)


def _prep_inputs(inputs):
    x = np.asarray(inputs["x"], np.float32)
    Wq_down = np.asarray(inputs["Wq_down"], np.float32)
    Wq_up = np.asarray(inputs["Wq_up"], np.float32)
    Wq_rope = np.asarray(inputs["Wq_rope"], np.float32)
    Wkv_down = np.asarray(inputs["Wkv_down"], np.float32)
    Wk_up = np.asarray(inputs["Wk_up"], np.float32)
    Wk_rope = np.asarray(inputs["Wk_rope"], np.float32)
    Wv_up = np.asarray(inputs["Wv_up"], np.float32)
    Wo = np.asarray(inputs["Wo"], np.float32)

    s = np.float32(1.0 / np.sqrt(DH))

    wd_kvT = np.ascontiguousarray(Wkv_down.T).astype(BF16NP)
    wd_qT = np.ascontiguousarray(Wq_down.T).astype(BF16NP)
    wkr2 = np.concatenate([Wk_rope, _rot_rows(Wk_rope)], axis=0)  # [64, HID]
    wkr2T = np.ascontiguousarray(wkr2.T).astype(BF16NP)

    inv_freq = (1.0 / (10000.0 ** (np.arange(0, R, 2, dtype=np.float32) / R)))
    t = np.arange(S, dtype=np.float32)
    freqs = t[:, None] * inv_freq[None, :]
    emb = np.concatenate([freqs, freqs], axis=-1)          # [S, R]
    cos4 = np.tile(np.cos(emb).T, (4, 1)).astype(BF16NP)   # [128, S]
    sin4 = np.tile(np.sin(emb).T, (4, 1)).astype(BF16NP)

    kar = np.arange(128)[:, None]
    qar = np.arange(512)[None, :]
    mask4 = np.empty((128, NQC * 512), np.float32)
    for off in range(4):
        mask4[:, off * 512:(off + 1) * 512] = (
            (128 * off + kar) <= qar
        ).astype(np.float32)
    mask4 = mask4.astype(BF16NP)

    per_g = []
    for g in range(4):
        hsl = slice(g * GH, (g + 1) * GH)
        wk_p = np.concatenate(
            [Wk_up[h * C:(h + 1) * C] for h in range(g * GH, (g + 1) * GH)],
            axis=0)                                         # [384, LAT]
        wv_p = np.concatenate(
            [Wv_up[h * DH:(h + 1) * DH] for h in range(g * GH, (g + 1) * GH)],
            axis=0)                                         # [512, LAT]
        wqc_p = np.concatenate(
            [Wq_up[h * C:(h + 1) * C] for h in range(g * GH, (g + 1) * GH)],
            axis=0) * s
        wqr_p = np.concatenate(
            [Wq_rope[h * R:(h + 1) * R] for h in range(g * GH, (g + 1) * GH)],
            axis=0) * s
        wqrr_p = np.concatenate(
            [_rot_rows(Wq_rope[h * R:(h + 1) * R])
             for h in range(g * GH, (g + 1) * GH)], axis=0) * s
        wo_g = Wo[:, g * GH * DH:(g + 1) * GH * DH]         # [HID, 512]
        per_g.append({
            "wk_pT": np.ascontiguousarray(wk_p.T).astype(BF16NP),
            "wv_pT": np.ascontiguousarray(wv_p.T).astype(BF16NP),
            "wqc_pT": np.ascontiguousarray(wqc_p.T).astype(BF16NP),
            "wqr_pT": np.ascontiguousarray(wqr_p.T).astype(BF16NP),
            "wqrr_pT": np.ascontiguousarray(wqrr_p.T).astype(BF16NP),
            "woT": np.ascontiguousarray(wo_g.T).astype(BF16NP),
        })

    in_maps = []
    for cid in range(8):
        b, g = divmod(cid, 4)
        tsl = slice(g * 512, (g + 1) * 512)
        m = {
            "xqT": np.ascontiguousarray(x[b, tsl].T).astype(BF16NP),
            "cosq": np.ascontiguousarray(cos4[0:32, tsl]),
            "sinq": np.ascontiguousarray(sin4[0:32, tsl]),
            "wd_kvT": wd_kvT, "wd_qT": wd_qT, "wkr2T": wkr2T,
            "cos4": cos4, "sin4": sin4, "mask4": mask4,
        }
        m.update(per_g[g])
        in_maps.append(m)
    return in_maps


_NC_CACHE = None


def kernel(**inputs):
    global _NC_CACHE
    if _NC_CACHE is None:
        _NC_CACHE = build_nc()
    nc = _NC_CACHE
    in_maps = _prep_inputs(inputs)
    res = run_bass_kernel_spmd(nc, in_maps, list(range(8)))
    bo = np.asarray(inputs["bo"], np.float32)
    out = np.empty((B, S, HID), np.float32)
    for b in range(B):
        acc = res.results[4 * b]["out"].astype(np.float32)
        for g in range(1, 4):
            acc = acc + res.results[4 * b + g]["out"]
        out[b] = acc + bo
    return out


# revision 48
# speedup vs baseline: 1.5436x; 1.0003x over previous
"""MLA (multi-head latent attention) Trainium2 kernel, SPMD over 8 NeuronCores.

Sharding: core c = 4*b + g handles batch b and head group g (4 heads),
ALL 2048 query rows.  Causality: query chunk c (512 rows) only attends
key chunks 0..4c+3 (lower triangle), so every core does the same
triangular work -- perfectly balanced, no masks off the diagonal.
Each core emits a PARTIAL out-projection (contraction over its 4 heads'
128-dims); the host sums the 4 partials per batch (+bias).  No
collectives.

On-chip layouts are transposed ([feature, token]) so every matmul
contracts over the partition dim with no on-chip transposes.
rotate_half is folded into host-permuted weight copies; 1/sqrt(dh) into
the q weights; softmax skips the max-pass (scores bounded) and gets its
row-sum from an all-ones matmul over a DVE tree-sum of the exp tiles.
Diagonal score tiles are masked multiplicatively (0/1 bf16) after exp.
"""

import os
import sys
import types

for _p in ("/opt/trn_rl_repo", "/root/.axon_site/_ro/trn_rl_repo"):
    if os.path.isdir(_p) and _p not in sys.path:
        sys.path.append(_p)

import numpy as np
import ml_dtypes

import concourse.bass as bass
import concourse.bacc as bacc_mod
import concourse.mybir as mybir
from concourse.tile import TileContext
from concourse.vector_clock import ScopedClock
from concourse.bass_utils import run_bass_kernel_spmd

F32 = mybir.dt.float32
BF16 = mybir.dt.bfloat16
BF16NP = ml_dtypes.bfloat16

HID, H, LAT, R, DH, C = 2048, 16, 512, 32, 128, 96
B, S = 2, 2048
GH = 4            # heads per core
NQC = 4           # query chunks of 512
NKC = 16          # key chunks of 128


def _patch_tile_drain():
    """The staged walrus rejects a Drain carrying >1 sync-wait. Move the
    TileContext tail-drain waits onto single-wait SP nops."""

    def _drain_and_barrier(self, tick_clock, wait_clock):
        drain_inst = self.nc.sync.drain()
        wait_clock.add_sem_waits(
            drain_inst.ins, ScopedClock({None: tick_clock.global_clock})
        )
        si = drain_inst.ins.sync_info
        if si is not None and len(si.on_wait) > 1:
            waits = list(si.on_wait)
            drain_inst.ins.sync_info = mybir.SyncInfo(
                on_wait=[], on_update=list(si.on_update)
            )
            for w in waits:
                nop = self.nc.sync.nop(nofuse=True)
                nop.ins.sync_info = mybir.SyncInfo(on_wait=[w], on_update=[])
        self.nc.all_engine_barrier()
        assert self.sems is not None
        popped = self.nc._tile_sem_poison_stack.pop()
        assert popped is self._sem_poison
        self.nc.clear_and_free_semaphores(list(self.sems.allocated().values()))
        self.nc.all_engine_barrier()

    TileContext._drain_and_barrier = _drain_and_barrier


def _install_ntff_hook():
    """antenv.axon_hooks is absent in this image; inject it and register the
    ctypes NTFF hook so trace=True / BASS_TRACE can profile."""
    try:
        import antenv

        if "antenv.axon_hooks" not in sys.modules:
            mod = types.ModuleType("antenv.axon_hooks")
            mod._hook = None

            def set_axon_ntff_profile_hook(h):
                mod._hook = h

            def get_axon_ntff_profile_hook():
                return mod._hook

            mod.set_axon_ntff_profile_hook = set_axon_ntff_profile_hook
            mod.get_axon_ntff_profile_hook = get_axon_ntff_profile_hook
            sys.modules["antenv.axon_hooks"] = mod
            antenv.axon_hooks = mod
        boot_dir = "/root/.axon_site/trn_agent_boot"
        so_path = "/opt/axon/libaxon_pjrt.so"
        if os.path.isdir(boot_dir) and os.path.exists(so_path):
            if boot_dir not in sys.path:
                sys.path.append(boot_dir)
            from trn_boot import _ntff_profile_via_ctypes

            hook = _ntff_profile_via_ctypes(so_path)
            if hook is not None:
                sys.modules["antenv.axon_hooks"].set_axon_ntff_profile_hook(hook)
    except Exception:
        pass


_patch_tile_drain()
_install_ntff_hook()


def _dram(nc, name, shape, dtype=F32, out=False):
    return nc.declare_dram_parameter(name, list(shape), dtype, isOutput=out)


def build_nc():
    nc = bacc_mod.Bacc("TRN2")

    xbT = _dram(nc, "xbT", [HID, S], BF16)            # x[b].T
    wd_kvT = _dram(nc, "wd_kvT", [HID, LAT], BF16)    # Wkv_down.T
    wd_qT = _dram(nc, "wd_qT", [HID, LAT], BF16)      # Wq_down.T
    wkr2T = _dram(nc, "wkr2T", [HID, 2 * R], BF16)    # [Wk_rope; rot].T
    wk_pT = _dram(nc, "wk_pT", [LAT, GH * C], BF16)   # 4-head k_c pack .T
    wv_pT = _dram(nc, "wv_pT", [LAT, GH * DH], BF16)  # 4-head v pack .T
    wqc_pT = _dram(nc, "wqc_pT", [LAT, GH * C], BF16)   # 4-head q_c pack /sqrt
    wqr_pT = _dram(nc, "wqr_pT", [LAT, GH * R], BF16)   # 4-head q_rope /sqrt
    wqrr_pT = _dram(nc, "wqrr_pT", [LAT, GH * R], BF16)  # rotated rope /sqrt
    woT = _dram(nc, "woT", [GH * DH, HID], BF16)      # Wo cols for our heads
    cos4_d = _dram(nc, "cos4", [128, S], BF16)        # cos.T tiled 4x
    sin4_d = _dram(nc, "sin4", [128, S], BF16)
    mask4_d = _dram(nc, "mask4", [128, NQC * 512], BF16)  # 0/1 diag masks
    out_d = _dram(nc, "out", [S, HID], out=True)      # partial (4-head) proj

    xbT_r = xbT[:, :].rearrange("(c p two) t -> c p two t", p=128, two=2)
    wd_kvT_r = wd_kvT[:, :].rearrange("(c p two) l -> c p two l", p=128, two=2)
    wd_qT_r = wd_qT[:, :].rearrange("(c p two) l -> c p two l", p=128, two=2)
    wkr2T_r = wkr2T[:, :].rearrange("(c p two) r -> c p two r", p=128, two=2)
    wk_pT_r = wk_pT[:, :].rearrange("(lc p) d -> lc p d", p=128)
    wv_pT_r = wv_pT[:, :].rearrange("(lc p) d -> lc p d", p=128)
    wqc_pT_r = wqc_pT[:, :].rearrange("(lc p) d -> lc p d", p=128)
    wqr_pT_r = wqr_pT[:, :].rearrange("(lc p) d -> lc p d", p=128)
    wqrr_pT_r = wqrr_pT[:, :].rearrange("(lc p) d -> lc p d", p=128)
    woT_r = woT[:, :].rearrange("(hc p) o -> hc p o", p=128)

    with TileContext(nc) as tc:
        with tc.tile_pool(name="perB", bufs=1) as perB, \
             tc.tile_pool(name="lat", bufs=2) as LATP, \
             tc.tile_pool(name="xs", bufs=1) as XS, \
             tc.tile_pool(name="ets", bufs=6) as ETS, \
             tc.tile_pool(name="acc", bufs=8) as ACC, \
             tc.tile_pool(name="rcp", bufs=2) as RCP, \
             tc.tile_pool(name="tmp", bufs=2) as TMP, \
             tc.tile_pool(name="ot", bufs=3) as OT, \
             tc.tile_pool(name="ps_g", bufs=2, space="PSUM") as PSG, \
             tc.tile_pool(name="ps_m", bufs=2, space="PSUM") as PSM, \
             tc.tile_pool(name="ps_s", bufs=2, space="PSUM") as PSS, \
             tc.tile_pool(name="ps_c", bufs=2, space="PSUM") as PSC:

            # ---------- persistent SBUF ----------
            krT = perB.tile([32, S], BF16, tag="krT", name="krT")
            kT = perB.tile([128, GH, S], BF16, tag="kT", name="kT")
            vG = perB.tile([128, NKC, GH * DH], BF16, tag="vG", name="vG")
            qT = perB.tile([128, GH, S], BF16, tag="qT", name="qT")
            ctxT = perB.tile([128, GH, S], BF16, tag="ctxT", name="ctxT")
            cos4 = perB.tile([128, S], BF16, tag="cos4", name="cos4")
            sin4 = perB.tile([128, S], BF16, tag="sin4", name="sin4")
            mask4 = perB.tile([128, NQC * 512], BF16, tag="mask4", name="mask4")
            onesb = perB.tile([128, 128], BF16, tag="ones", name="ones")
            wk_sb = perB.tile([128, 4, GH * C], BF16, tag="wk", name="wk")
            wv_sb = perB.tile([128, 4, GH * DH], BF16, tag="wv", name="wv")
            wqc_sb = perB.tile([128, 4, GH * C], BF16, tag="wqc", name="wqc")
            wqr_sb = perB.tile([128, 4, GH * R], BF16, tag="wqr", name="wqr")
            wqrr_sb = perB.tile([128, 4, GH * R], BF16, tag="wqrr", name="wqrr")

            # down-proj weights: released after phase A(3), wo loaded after.
            # Per-hc tiles so the first matmuls wait only on their own slice;
            # kv weights first (the very first accumulation pass).
            WD = tc.alloc_tile_pool(name="wd", bufs=1, side="right")
            wdkv = [WD.tile([128, 2, LAT], BF16, tag=f"wdkv{hc}",
                            name=f"wdkv{hc}") for hc in range(8)]
            wdq = [WD.tile([128, 2, LAT], BF16, tag=f"wdq{hc}",
                           name=f"wdq{hc}") for hc in range(8)]
            wkr = [WD.tile([128, 2, 2 * R], BF16, tag=f"wkr{hc}",
                           name=f"wkr{hc}") for hc in range(8)]

            def load_wd():
                for hc in range(8):
                    nc.sync.dma_start(wdkv[hc][:], wd_kvT_r[hc])
                for hc in range(8):
                    nc.sync.dma_start(wkr[hc][:], wkr2T_r[hc])
                    nc.sync.dma_start(wdq[hc][:], wd_qT_r[hc])

            def load_x(tq):
                tsl = slice(tq * 512, (tq + 1) * 512)
                xt = [XS.tile([128, 2, 512], BF16, tag=f"xf{hc}",
                              name=f"xf{hc}") for hc in range(8)]
                for hc in range(8):
                    nc.sync.dma_start(xt[hc][:], xbT_r[hc][:, :, tsl])
                return xt

            WO = [None]  # box for the late wo pool
            wo_sb = [None]

            def load_cossin():
                nc.sync.dma_start(cos4[:], cos4_d[:, :])
                nc.sync.dma_start(sin4[:], sin4_d[:, :])

            def load_aux_weights():
                nc.sync.dma_start(mask4[:], mask4_d[:, :])
                nc.gpsimd.memset(onesb[:], 1.0)
                for lc in range(4):
                    nc.sync.dma_start(wk_sb[:, lc, :], wk_pT_r[lc])
                    nc.sync.dma_start(wv_sb[:, lc, :], wv_pT_r[lc])
                    nc.sync.dma_start(wqc_sb[:, lc, :], wqc_pT_r[lc])
                    nc.sync.dma_start(wqr_sb[:, lc, :], wqr_pT_r[lc])
                    nc.sync.dma_start(wqrr_sb[:, lc, :], wqrr_pT_r[lc])

            # ---------------- phase emitters ----------------
            def phA(tq, xt=None):
                """latents for token quarter tq: kv_lat, roped k_rope, q_lat.
                Returns the per-quarter latent tiles for phB(tq)."""
                tsl = slice(tq * 512, (tq + 1) * 512)
                if xt is None:
                    xt = load_x(tq)
                kv_t = LATP.tile([128, 4, 512], BF16, tag="kvlat",
                                 name="kvlat")
                q_t = LATP.tile([128, 4, 512], BF16, tag="qlat", name="qlat")

                # kv_lat: 4 lc passes, 2 rotating psum banks
                for lc in range(4):
                    ps = PSG.tile([128, 512], F32, tag="g", name=f"pkv{lc}")
                    for hc in range(8):
                        for two in range(2):
                            nc.tensor.matmul(
                                ps[:],
                                lhsT=wdkv[hc][:, two, lc * 128:(lc + 1) * 128],
                                rhs=xt[hc][:, two, :],
                                start=(hc == 0 and two == 0),
                                stop=(hc == 7 and two == 1),
                            )
                    nc.vector.tensor_copy(kv_t[:, lc, :], ps[:])
                # k_rope pass (64 rows: [rope; rot]); combine in place
                pkr = PSG.tile([64, 512], F32, tag="g", name="pkr")
                for hc in range(8):
                    for two in range(2):
                        nc.tensor.matmul(
                            pkr[:],
                            lhsT=wkr[hc][:, two, :],
                            rhs=xt[hc][:, two, :],
                            start=(hc == 0 and two == 0),
                            stop=(hc == 7 and two == 1),
                        )
                nc.vector.tensor_mul(pkr[0:32, :], pkr[0:32, :],
                                     cos4[0:32, tsl])
                tkr = TMP.tile([32, 512], F32, tag="tkr", name="tkr")
                nc.vector.tensor_mul(tkr[:], pkr[32:64, :], sin4[0:32, tsl])
                nc.vector.tensor_add(krT[:, tsl], pkr[0:32, :], tkr[:])
                # q_lat: 4 lc passes
                for lc in range(4):
                    ps = PSG.tile([128, 512], F32, tag="g", name=f"pq{lc}")
                    for hc in range(8):
                        for two in range(2):
                            nc.tensor.matmul(
                                ps[:],
                                lhsT=wdq[hc][:, two, lc * 128:(lc + 1) * 128],
                                rhs=xt[hc][:, two, :],
                                start=(hc == 0 and two == 0),
                                stop=(hc == 7 and two == 1),
                            )
                    nc.vector.tensor_copy(q_t[:, lc, :], ps[:])
                return kv_t, q_t

            def phB_k(tq, kv_t):
                """k_c per head (96 content rows) + shared roped k_rope."""
                tsl = slice(tq * 512, (tq + 1) * 512)
                for h in range(GH):
                    ps = PSM.tile([128, 512], F32, tag="m", name=f"pk{h}")
                    for lc in range(4):
                        nc.tensor.matmul(
                            ps[0:C, :],
                            lhsT=wk_sb[:, lc, h * C:(h + 1) * C],
                            rhs=kv_t[:, lc, :],
                            start=(lc == 0), stop=(lc == 3),
                        )
                    nc.vector.tensor_copy(kT[0:C, h, tsl], ps[0:C, :])
                for h in range(GH):
                    nc.sync.dma_start(kT[C:128, h, tsl], krT[:, tsl])

            def phB_v(tq, kv_t):
                """v: 4 token sub-chunks of 128, out = [t, 4h*128]."""
                for t2 in range(4):
                    kc = tq * 4 + t2
                    ps = PSM.tile([128, 512], F32, tag="m", name=f"pv{t2}")
                    for lc in range(4):
                        nc.tensor.matmul(
                            ps[:],
                            lhsT=kv_t[:, lc, t2 * 128:(t2 + 1) * 128],
                            rhs=wv_sb[:, lc, :],
                            start=(lc == 0), stop=(lc == 3),
                        )
                    nc.vector.tensor_copy(vG[:, kc, :], ps[:])

            def phB_qc(tq, q_t):
                """q_c per head."""
                tsl = slice(tq * 512, (tq + 1) * 512)
                for h in range(GH):
                    ps = PSM.tile([128, 512], F32, tag="m", name=f"pqc{h}")
                    for lc in range(4):
                        nc.tensor.matmul(
                            ps[0:C, :],
                            lhsT=wqc_sb[:, lc, h * C:(h + 1) * C],
                            rhs=q_t[:, lc, :],
                            start=(lc == 0), stop=(lc == 3),
                        )
                    nc.vector.tensor_copy(qT[0:C, h, tsl], ps[0:C, :])

            def phB_qr(tq, q_t):
                """q_rope: stacked 4h x 32 rope + rot; combine, scatter."""
                tsl = slice(tq * 512, (tq + 1) * 512)
                psr = PSM.tile([128, 512], F32, tag="m", name="pqr")
                psrr = PSM.tile([128, 512], F32, tag="m", name="pqrr")
                for lc in range(4):
                    nc.tensor.matmul(
                        psr[:], lhsT=wqr_sb[:, lc, :],
                        rhs=q_t[:, lc, :],
                        start=(lc == 0), stop=(lc == 3),
                    )
                for lc in range(4):
                    nc.tensor.matmul(
                        psrr[:], lhsT=wqrr_sb[:, lc, :],
                        rhs=q_t[:, lc, :],
                        start=(lc == 0), stop=(lc == 3),
                    )
                t2b = TMP.tile([128, 512], F32, tag="t2b", name="t2b")
                t3 = TMP.tile([128, 512], BF16, tag="t3b", name="t3b")
                nc.vector.tensor_mul(psr[:], psr[:], cos4[:, tsl])
                nc.vector.tensor_mul(t2b[:], psrr[:], sin4[:, tsl])
                nc.vector.tensor_add(t3[:], psr[:], t2b[:])
                for h in range(GH):
                    nc.sync.dma_start(
                        qT[C:128, h, tsl], t3[32 * h:32 * h + 32, :]
                    )

            def phC_head(c, h):
                """attention main for (chunk c, head h): scores+exp+ctx+tree.
                Returns state for phC_fin.  Diagonal key chunks last so their
                mask-multiply stays off the exp->ctx critical path."""
                csl = slice(c * 512, (c + 1) * 512)
                nkc = 4 * (c + 1)
                kcs = list(range(4 * c)) + [4 * c + d for d in range(4)]
                ets = {}
                stack = []  # binary-counter tree: list of (level, tile)

                def emit_score(kc, i):
                    ps = PSS.tile([128, 512], F32, tag="s", name=f"ps{i % 2}")
                    nc.tensor.matmul(
                        ps[:],
                        lhsT=kT[:, h, kc * 128:(kc + 1) * 128],
                        rhs=qT[:, h, csl],
                        start=True, stop=True,
                    )
                    et = ETS.tile([128, 512], BF16, tag="e", name=f"et{i % 6}")
                    nc.scalar.activation(
                        et[:], ps[:], mybir.ActivationFunctionType.Exp
                    )
                    off = kc - 4 * c
                    if off >= 0:
                        nc.vector.tensor_mul(
                            et[:], et[:], mask4[:, off * 512:(off + 1) * 512]
                        )
                    ets[kc] = et

                pctx = PSC.tile([128, 512], F32, tag="c", name="pctx")

                def emit_ctx(kc, i):
                    nc.tensor.matmul(
                        pctx[:],
                        lhsT=vG[:, kc, h * DH:(h + 1) * DH],
                        rhs=ets[kc][:],
                        start=(i == 0), stop=(i == nkc - 1),
                    )
                    # fold into the tree-sum (DVE, bf16)
                    carry = ets[kc]
                    lvl = 0
                    while stack and stack[-1][0] == lvl:
                        _, other = stack.pop()
                        dst = ACC.tile([128, 512], BF16, tag="a",
                                       name=f"acc{i % 5}")
                        nc.vector.tensor_add(dst[:], other[:], carry[:])
                        carry = dst
                        lvl += 1
                    stack.append((lvl, carry))

                LAG = 3
                for i, kc in enumerate(kcs):
                    emit_score(kc, i)
                    if i >= LAG:
                        emit_ctx(kcs[i - LAG], i - LAG)
                for i in range(max(0, nkc - LAG), nkc):
                    emit_ctx(kcs[i], i)
                # fold remaining tree levels
                while len(stack) > 1:
                    l1, a = stack.pop()
                    l2, b = stack.pop()
                    dst = ACC.tile([128, 512], BF16, tag="a", name="accf")
                    nc.vector.tensor_add(dst[:], a[:], b[:])
                    stack.append((max(l1, l2) + 1, dst))
                return pctx, stack[0][1]

            def phC_fin(c, h, pctx, tsum):
                """row-sum via all-ones matmul, reciprocal, ctx normalize."""
                csl = slice(c * 512, (c + 1) * 512)
                prs = PSM.tile([128, 512], F32, tag="m", name="prs")
                nc.tensor.matmul(
                    prs[:], lhsT=onesb[:], rhs=tsum[:], start=True, stop=True
                )
                rc = RCP.tile([128, 512], F32, tag="rc", name="rc")
                nc.vector.reciprocal_approx_fast(out=rc[:], in_=prs[:])
                nc.vector.tensor_mul(ctxT[:, h, csl], pctx[:], rc[:])

            def phC(c):
                for h in range(GH):
                    pctx, tsum = phC_head(c, h)
                    phC_fin(c, h, pctx, tsum)

            def load_wo():
                if WO[0] is None:
                    WO[0] = tc.alloc_tile_pool(name="wo", bufs=1, side="right")
                    wo_sb[0] = WO[0].tile([128, 4, HID], BF16, tag="wo",
                                          name="wo")
                    for hc in range(4):
                        nc.sync.dma_start(wo_sb[0][:, hc, :], woT_r[hc])

            def phD_qb(qb):
                """partial out-projection for one 128-row query block."""
                for oc in range(4):
                    ps = PSM.tile([128, 512], F32, tag="m",
                                  name=f"po{oc % 3}")
                    for h in range(GH):
                        nc.tensor.matmul(
                            ps[:],
                            lhsT=ctxT[:, h, qb * 128:(qb + 1) * 128],
                            rhs=wo_sb[0][:, h, oc * 512:(oc + 1) * 512],
                            start=(h == 0), stop=(h == 3),
                        )
                    ot = OT.tile([128, 512], F32, tag="ot", name="ot")
                    nc.vector.tensor_copy(ot[:], ps[:])
                    nc.sync.dma_start(
                        out_d[qb * 128:(qb + 1) * 128,
                              oc * 512:(oc + 1) * 512],
                        ot[:],
                    )

            # ---------------- master schedule ----------------
            xt0 = load_x(0)
            load_wd()
            load_cossin()
            # warm-up matmuls on a memset tile: PE starts at ~+2us (no DMA
            # dependency), so HAM is un-throttled before the real work
            wmt = perB.tile([128, 512], BF16, tag="wmt", name="wmt")
            nc.gpsimd.memset(wmt[:], 0.0)
            for i in range(12):
                pw = PSM.tile([128, 512], F32, tag="m", name=f"warm{i % 2}")
                nc.tensor.matmul(pw[:], lhsT=wmt[:, 0:128], rhs=wmt[:],
                                 start=True, stop=True)
            lat0 = phA(0, xt0)
            load_aux_weights()
            phB_k(0, lat0[0])
            phB_v(0, lat0[0])
            phB_qc(0, lat0[1])
            phB_qr(0, lat0[1])
            lat1 = phA(1)

            def run_chunk(c, fillers):
                for h in range(GH):
                    st = phC_head(c, h)
                    for f in fillers[h]:
                        f()
                    phC_fin(c, h, *st)

            # per-head fillers keep the PE fed while ACT does the exps
            run_chunk(0, [
                [lambda: phB_k(1, lat1[0])],
                [lambda: phB_v(1, lat1[0])],
                [lambda: phB_qc(1, lat1[1])],
                [lambda: phB_qr(1, lat1[1])],
            ])
            lat2 = phA(2)
            run_chunk(1, [
                [lambda: phB_k(2, lat2[0])],
                [lambda: phB_v(2, lat2[0])],
                [lambda: phB_qc(2, lat2[1])],
                [lambda: phB_qr(2, lat2[1])],
            ])
            lat3 = phA(3)
            WD.release()
            load_wo()
            run_chunk(2, [
                [lambda: phB_k(3, lat3[0])],
                [lambda: phB_v(3, lat3[0])],
                [lambda: phB_qc(3, lat3[1])],
                [lambda: phB_qr(3, lat3[1]), lambda: phD_qb(0)],
            ])
            run_chunk(3, [
                [lambda: phD_qb(1), lambda: phD_qb(2)],
                [lambda: phD_qb(3), lambda: phD_qb(4)],
                [lambda: phD_qb(5), lambda: phD_qb(6)],
                [lambda: phD_qb(7), lambda: phD_qb(8)],
            ])
            for qb in range(9, 16):
                phD_qb(qb)
            if WO[0] is not None:
                WO[0].release()

    nc.compile()
    return nc


def _rot_rows(w):
    # rows of w are the rope dim; rot(w) @ lat == rotate_half(w @ lat)
    hR = w.shape[0] // 2
    return np.concatenate([-w[hR:], w[:hR]], axis=0)


def _prep_inputs(inputs):
    x = np.asarray(inputs["x"], np.float32)
    Wq_down = np.asarray(inputs["Wq_down"], np.float32)
    Wq_up = np.asarray(inputs["Wq_up"], np.float32)
    Wq_rope = np.asarray(inputs["Wq_rope"], np.float32)
    Wkv_down = np.asarray(inputs["Wkv_down"], np.float32)
    Wk_up = np.asarray(inputs["Wk_up"], np.float32)
    Wk_rope = np.asarray(inputs["Wk_rope"], np.float32)
    Wv_up = np.asarray(inputs["Wv_up"], np.float32)
    Wo = np.asarray(inputs["Wo"], np.float32)

    s = np.float32(1.0 / np.sqrt(DH))

    wd_kvT = np.ascontiguousarray(Wkv_down.T).astype(BF16NP)
    wd_qT = np.ascontiguousarray(Wq_down.T).astype(BF16NP)
    wkr2 = np.concatenate([Wk_rope, _rot_rows(Wk_rope)], axis=0)  # [64, HID]
    wkr2T = np.ascontiguousarray(wkr2.T).astype(BF16NP)

    inv_freq = (1.0 / (10000.0 ** (np.arange(0, R, 2, dtype=np.float32) / R)))
    t = np.arange(S, dtype=np.float32)
    freqs = t[:, None] * inv_freq[None, :]
    emb = np.concatenate([freqs, freqs], axis=-1)          # [S, R]
    cos4 = np.tile(np.cos(emb).T, (4, 1)).astype(BF16NP)   # [128, S]
    sin4 = np.tile(np.sin(emb).T, (4, 1)).astype(BF16NP)

    kar = np.arange(128)[:, None]
    qar = np.arange(512)[None, :]
    mask4 = np.empty((128, NQC * 512), np.float32)
    for off in range(4):
        mask4[:, off * 512:(off + 1) * 512] = (
            (128 * off + kar) <= qar
        ).astype(np.float32)
    mask4 = mask4.astype(BF16NP)

    per_g = []
    for g in range(4):
        hsl = slice(g * GH, (g + 1) * GH)
        wk_p = np.concatenate(
            [Wk_up[h * C:(h + 1) * C] for h in range(g * GH, (g + 1) * GH)],
            axis=0)                                         # [384, LAT]
        wv_p = np.concatenate(
            [Wv_up[h * DH:(h + 1) * DH] for h in range(g * GH, (g + 1) * GH)],
            axis=0)                                         # [512, LAT]
        wqc_p = np.concatenate(
            [Wq_up[h * C:(h + 1) * C] for h in range(g * GH, (g + 1) * GH)],
            axis=0) * s
        wqr_p = np.concatenate(
            [Wq_rope[h * R:(h + 1) * R] for h in range(g * GH, (g + 1) * GH)],
            axis=0) * s
        wqrr_p = np.concatenate(
            [_rot_rows(Wq_rope[h * R:(h + 1) * R])
             for h in range(g * GH, (g + 1) * GH)], axis=0) * s
        wo_g = Wo[:, g * GH * DH:(g + 1) * GH * DH]         # [HID, 512]
        per_g.append({
            "wk_pT": np.ascontiguousarray(wk_p.T).astype(BF16NP),
            "wv_pT": np.ascontiguousarray(wv_p.T).astype(BF16NP),
            "wqc_pT": np.ascontiguousarray(wqc_p.T).astype(BF16NP),
            "wqr_pT": np.ascontiguousarray(wqr_p.T).astype(BF16NP),
            "wqrr_pT": np.ascontiguousarray(wqrr_p.T).astype(BF16NP),
            "woT": np.ascontiguousarray(wo_g.T).astype(BF16NP),
        })

    in_maps = []
    for cid in range(8):
        b, g = divmod(cid, 4)
        m = {
            "xbT": np.ascontiguousarray(x[b].T).astype(BF16NP),
            "wd_kvT": wd_kvT, "wd_qT": wd_qT, "wkr2T": wkr2T,
            "cos4": cos4, "sin4": sin4, "mask4": mask4,
        }
        m.update(per_g[g])
        in_maps.append(m)
    return in_maps


_NC_CACHE = None


def kernel(**inputs):
    global _NC_CACHE
    if _NC_CACHE is None:
        _NC_CACHE = build_nc()
    nc = _NC_CACHE
    in_maps = _prep_inputs(inputs)
    res = run_bass_kernel_spmd(nc, in_maps, list(range(8)))
    bo = np.asarray(inputs["bo"], np.float32)
    out = np.empty((B, S, HID), np.float32)
    for b in range(B):
        acc = res.results[4 * b]["out"].astype(np.float32)
        for g in range(1, 4):
            acc = acc + res.results[4 * b + g]["out"]
        out[b] = acc + bo
    return out
